# revision 57
# baseline (speedup 1.0000x reference)
"""Dense transformer block on 8 TRN2 NeuronCores.

Sharding: data-parallel over batch (4 pairs of cores). Within each pair:
  - Attention is Megatron head-parallel (8 heads per core, all tokens).
  - Post-attention (residual+LN1+MLP+LN2) is chunk-parallel: partial sums
    of the attention projection for two 512-token chunks are combined with
    one pairwise ReduceScatter per chunk-pair; core r owns chunks {r, 2+r}
    and runs the MLP full-width locally (no second collective).

Device schedule is built around keeping the PE (tensor engine)
continuously busy so it stays at its max p-state clock:
  - Scores for both heads of a (krt, t) unit land in ONE 2-bank PSUM tile
    [128, 1024] and are exponentiated by a single wide ACT instruction.
  - The causal mask is applied by a tiny extra matmul (tri_neg @ I128)
    accumulated into the score PSUM *before* the exp, so there is no
    vector-engine mask pass between exp and attn@V.
  - The PE stream is software-pipelined one unit ahead (S(u+1) issues
    before A(u)), and QKV matmuls of chunk c+1 plus the projection of
    chunk c are woven into attention of chunk c as filler so the PE never
    waits on the Scalar engine's exp stream.
  - Softmax denominators ride attn@V as an extra ones-column of V; the
    two den rows are gathered to partitions 0..1 with a tiny DMA,
    reciprocated with the fast DVE approx, and broadcast across
    partitions with one sel2 @ rec matmul whose emission is deferred a
    few units so the PE stream never waits on the den chain.
  - RS-gated work (residual loads/adds after each ReduceScatter) runs on
    the otherwise idle gpsimd queue, and tile_wait_until hints keep the
    scheduler from hoisting its consumers into engine-queue slots where
    they would head-of-line block attention.
"""

import numpy as np
import ml_dtypes

import concourse.bacc as bacc
import concourse.mybir as mybir
import concourse.tile as tile
from concourse.bass_utils import run_bass_kernel_spmd

F32 = mybir.dt.float32
BF16 = mybir.dt.bfloat16
AF = mybir.ActivationFunctionType
OP = mybir.AluOpType

B, S, D, H, HD, FF = 4, 2048, 1024, 16, 64, 4096
N_CORES = 8
PAIRS = [[0, 1], [2, 3], [4, 5], [6, 7]]
CH = 512                 # tokens per chunk
NCH = S // CH            # 4
DT = D // 128            # 8 d-tiles
FT = FF // 128           # 32 f-tiles
KT = S // 128            # 16 kpos tiles
EPS = 1e-5
BF = ml_dtypes.bfloat16

PE_NS = 0.42             # ns per output column at max clock
EXP_NS = 0.833           # scalar ns per column


def _build(use_bqk, use_bv, use_projb, use_cprojb, use_g1b1, use_g2b2):
    nc = bacc.Bacc("TRN2", target_bir_lowering=False, debug=False,
                   enable_asserts=True, num_devices=N_CORES)

    # ---- DRAM inputs (tile-packed on host) ----
    xqb = nc.dram_tensor("xqb", [NCH, 2, 128, 4 * 512], BF16,
                         kind="ExternalInput")          # bf16 x^T (c, half)
    xo = nc.dram_tensor("xo", [2 * DT, 128, 512], F32,
                        kind="ExternalInput")           # f32 x^T own chunks
    wqk = nc.dram_tensor("wqk", [128, 16 * 512], BF16, kind="ExternalInput")
    bqk = nc.dram_tensor("bqk", [1024], F32, kind="ExternalInput")
    wv = nc.dram_tensor("wv", [128, 8 * 512], BF16, kind="ExternalInput")
    bv = nc.dram_tensor("bv", [512], BF16, kind="ExternalInput")
    wproj = nc.dram_tensor("wproj", [128, 8 * 512], BF16,
                           kind="ExternalInput")
    projb = nc.dram_tensor("projb", [D], F32, kind="ExternalInput")
    wfc = nc.dram_tensor("wfc", [8, 2, 128, 4 * 512], BF16,
                         kind="ExternalInput")          # (fg, half) x (d,q)
    fcb = nc.dram_tensor("fcb", [FF], F32, kind="ExternalInput")
    wcp = nc.dram_tensor("wcp", [4, 4, 128, 8 * 256], BF16,
                         kind="ExternalInput")          # (p4, qtr) x (f,q)
    cprojb = nc.dram_tensor("cprojb", [D], F32, kind="ExternalInput")
    g1 = nc.dram_tensor("g1", [D], F32, kind="ExternalInput")
    b1 = nc.dram_tensor("b1", [D], F32, kind="ExternalInput")
    g2 = nc.dram_tensor("g2", [D], F32, kind="ExternalInput")
    b2 = nc.dram_tensor("b2", [D], F32, kind="ExternalInput")
    lna = nc.dram_tensor("lna", [S], F32, kind="ExternalInput")
    trin = nc.dram_tensor("trin", [128, 128], BF16, kind="ExternalInput")
    eye = nc.dram_tensor("eye", [128, 128], BF16, kind="ExternalInput")
    sel2 = nc.dram_tensor("sel2", [1, 256], BF16, kind="ExternalInput")
    # output: own chunks (k, i) tiles; host reassembles
    out = nc.dram_tensor("out", [2 * DT, 128, 512], F32,
                         kind="ExternalOutput")

    from contextlib import ExitStack
    with tile.TileContext(nc) as tc, ExitStack() as ctx:
        def pool(name, bufs, space="SBUF"):
            return ctx.enter_context(
                tc.tile_pool(name=name, bufs=bufs, space=space))

        const = pool("const", 1)
        xb_p = pool("xb_p", 2)          # bf16 x half-chunks [128, 2048]
        qTb_p = pool("qTb_p", 6)
        pt_p = pool("pt_p", 4)          # exp outputs [128, 1024] bf16
        attnTb_p = pool("attnTb_p", 8)
        dd_p = pool("dd_p", 2)          # den recip rows f32
        ddb_p = pool("ddb_p", 2)        # den recip rows bf16
        t64_p = pool("t64_p", 2)        # odd-head attn staging for DMA move
        ai_p = pool("ai_p", 2)          # proj partial bf16 tiles
        rob_p = pool("rob_p", 2)        # bf16 rs_out staging
        t1_p = pool("t1_p", 10)         # B: residual tiles f32 (t1 AND n+m)
        xf2_p = pool("xf2_p", 2)
        cast_p = pool("cast_p", 1)      # LN bf16 casts
        sq_p = pool("sq_p", 1)
        strow_p = pool("strow_p", 1)
        nTb_p = pool("nTb_p", 8)        # bf16 n tiles (fc rhs + s3 residual)
        tmpn_p = pool("tmpn_p", 1)
        hT_p = pool("hT_p", 3)
        ubc_p = pool("ubc_p", 1)        # u*rstd SBUF copy
        psS = pool("psS", 2, "PSUM")    # [128,1024] score pair tiles
        psA = pool("psA", 2, "PSUM")    # [128,512] av accumulators + bcast
        psM = pool("psM", 2, "PSUM")    # [128,512] general matmul tiles
        dram = pool("dram", 2, "DRAM")

        # ---- constants ----
        kt_sb = const.tile([128, 4 * S], BF16, name="kt_sb")
        kt_v = kt_sb[:].rearrange("p (r q) -> p r q", q=S)
        v_sb = const.tile([128, KT * 520], BF16, name="v_sb")
        v_v = v_sb[:].rearrange("p (t e) -> p t e", e=520)
        # ones column of V (den rides attn@V as the 65th row), set once
        v_h = v_sb[:].rearrange("p (t h e) -> p t h e", h=8, e=65)
        nc.vector.memset(v_h[:, :, :, 64:65], 1.0)

        # prefetch chunk 0's x before the big weight DMAs
        xh0 = []
        for half in range(2):
            t_ = xb_p.tile([128, 4 * 512], BF16, name="xh0")
            nc.sync.dma_start(out=t_[:], in_=xqb[0, half])
            xh0.append(t_)

        # ---- resident attention weights (pool closed before B phase so
        # the MLP weight pools can reuse its SBUF space) ----
        actx = ExitStack()
        wres = actx.enter_context(
            tc.tile_pool(name="wres", bufs=1, space="SBUF"))
        wqk_sb = wres.tile([128, 16 * 512], BF16, name="wqk_sb")
        nc.sync.dma_start(out=wqk_sb[:, 0:8 * 512], in_=wqk[:, 0:8 * 512])
        nc.scalar.dma_start(out=wqk_sb[:, 8 * 512:], in_=wqk[:, 8 * 512:])
        wqk_t = [wqk_sb[:, 512 * i:512 * (i + 1)] for i in range(16)]
        wv_sb = wres.tile([128, 8 * 512], BF16, name="wv_sb")
        nc.scalar.dma_start(out=wv_sb[:], in_=wv[:])
        wv_t = [wv_sb[:, 512 * i:512 * (i + 1)] for i in range(8)]
        wpr_sb = wres.tile([128, 8 * 512], BF16, name="wpr_sb")
        nc.scalar.dma_start(out=wpr_sb[:], in_=wproj[:])
        wpr_t = [wpr_sb[:, 512 * i:512 * (i + 1)] for i in range(8)]

        trin_sb = const.tile([128, 128], BF16, name="trin_sb")
        nc.scalar.dma_start(out=trin_sb[:], in_=trin[:])
        eye_sb = const.tile([128, 128], BF16, name="eye_sb")
        nc.scalar.dma_start(out=eye_sb[:], in_=eye[:])
        sel2b_sb = const.tile([2, 128], BF16, name="sel2b_sb")
        nc.scalar.dma_start(out=sel2b_sb[:],
                            in_=sel2.rearrange("o (a b) -> (o a) b", a=2))
        lna_sb = const.tile([128, KT], F32, name="lna_sb")
        nc.scalar.dma_start(out=lna_sb[:],
                            in_=lna.rearrange("(t p) -> p t", p=128))
        ones_col_b = const.tile([128, 1], BF16, name="ones_col_b")
        nc.vector.memset(ones_col_b[:], 1.0)
        ones_row_f = const.tile([1, 128], F32, name="ones_row_f")
        nc.vector.memset(ones_row_f[:], 1.0)
        eps_sb = const.tile([1, 1], F32, name="eps_sb")
        nc.vector.memset(eps_sb[:], EPS)
        fcb_sb = const.tile([128, FT], F32, name="fcb_sb")
        nc.scalar.dma_start(out=fcb_sb[:],
                            in_=fcb.rearrange("(i p) -> p i", p=128))

        def vec8(name, t):
            sb = const.tile([128, DT], F32, name=name)
            nc.scalar.dma_start(out=sb[:],
                                in_=t.rearrange("(i p) -> p i", p=128))
            return sb

        bqk_sb = vec8("bqk_sb", bqk) if use_bqk else None
        projb_sb = vec8("projb_sb", projb) if use_projb else None
        cprojb_sb = vec8("cprojb_sb", cprojb) if use_cprojb else None
        g1_sb = vec8("g1_sb", g1) if use_g1b1 else None
        b1_sb = vec8("b1_sb", b1) if use_g1b1 else None
        g2_sb = vec8("g2_sb", g2) if use_g2b2 else None
        b2_sb = vec8("b2_sb", b2) if use_g2b2 else None
        if use_bv:
            ones_row_b = const.tile([1, 128], BF16, name="ones_row_b")
            nc.vector.memset(ones_row_b[:], 1.0)
            bv_sb = const.tile([1, 512], BF16, name="bv_sb")
            nc.sync.dma_start(out=bv_sb[:],
                              in_=bv.rearrange("(o q) -> o q", o=1))

        # ---- ReduceScatter buffers (bf16 payload halves the wire) ----
        rs_in = [dram.tile([2 * D, 512], BF16, tag=f"rsi{j}",
                           name=f"rs_in{j}") for j in range(2)]
        rs_out = [dram.tile([D, 512], BF16, tag=f"rso{j}",
                            name=f"rs_out{j}") for j in range(2)]

        qtb_tiles = {0: [None] * 4, 1: [None] * 4, 2: [None] * 4,
                     3: [None] * 4}
        attn_tiles = {c: [None] * 4 for c in range(4)}

        # ================= QKV filler steps for chunk c ====================
        # Steps are (pe_cost_ns, closure, chain_start).  chain_start marks
        # safe points where a deferred rb matmul may allocate from the
        # shared psM ring without colliding with an open accumulation chain.
        def qkv_steps(c):
            tok = slice(CH * c, CH * (c + 1))
            steps = []
            if c == 0:
                xh = xh0
            else:
                xh = [None, None]

                def load(half):
                    def f():
                        t_ = xb_p.tile([128, 4 * 512], BF16, name="xh")
                        nc.sync.dma_start(out=t_[:], in_=xqb[c, half])
                        xh[half] = t_
                    return f
                steps.append((0, load(0), True))
                steps.append((0, load(1), True))

            def xtb(d):
                return xh[d // 4][:, 512 * (d % 4):512 * (d % 4 + 1)]

            # 8 QK chains (i<4: Q -> qTb, else K -> kt_v)
            for cc in range(2):
                for ct in range(4):
                    i = 4 * cc + ct
                    box = [None]

                    def mk(d, i=i, cc=cc, ct=ct, box=box):
                        def f():
                            if d == 0:
                                box[0] = psM.tile([128, 512], F32, tag="mm",
                                                  name="ps_qk")
                            nc.tensor.matmul(
                                box[0][:],
                                wqk_t[8 * cc + d][:, 128 * ct:128 * (ct + 1)],
                                xtb(d), start=(d == 0), stop=(d == DT - 1))
                            if d == DT - 1:
                                if i < 4:
                                    dt_ = qTb_p.tile([128, 512], BF16,
                                                     name="qTb")
                                    qtb_tiles[c][i] = dt_
                                    dest = dt_[:]
                                else:
                                    dest = kt_v[:, i - 4, tok]
                                if use_bqk:
                                    nc.vector.tensor_scalar_add(
                                        dest, box[0][:], bqk_sb[:, i:i + 1])
                                else:
                                    nc.vector.tensor_copy(dest, box[0][:])
                        return f
                    for d in range(DT):
                        steps.append((512 * PE_NS, mk(d), d == 0))

            # 4 V chains
            for tt in range(4):
                tg = 4 * c + tt
                box = [None]

                def mkv(d, tt=tt, tg=tg, box=box):
                    def f():
                        if d == 0:
                            box[0] = psM.tile([128, 512], F32, tag="mm",
                                              name="ps_v")
                        nc.tensor.matmul(
                            box[0][:], xtb(d)[:, 128 * tt:128 * (tt + 1)],
                            wv_t[d], start=(d == 0),
                            stop=(d == DT - 1 and not use_bv))
                        if d == DT - 1:
                            if use_bv:
                                nc.tensor.matmul(box[0][:], ones_row_b[:],
                                                 bv_sb[:], start=False,
                                                 stop=True)
                            ps_h = box[0][:].rearrange(
                                "p (h e) -> p h e", e=64)
                            nc.vector.tensor_copy(
                                v_h[:, tg, :, 0:64], ps_h[:])
                    return f
                for d in range(DT):
                    steps.append((512 * PE_NS, mkv(d), d == 0))
            return steps

        # ================= proj steps for chunk c ==========================
        def proj_steps(c, drain_scalar=False):
            blk = c % 2
            ri = rs_in[c // 2][:] \
                .rearrange("(k i p) q -> k i p q", k=2, p=128)
            steps = []
            for cc in range(2):
                for ct in range(4):
                    dct = 4 * cc + ct
                    box = [None]

                    def mk(r, cc=cc, ct=ct, dct=dct, box=box):
                        def f():
                            if r == 0:
                                box[0] = psM.tile([128, 512], F32, tag="mm",
                                                  name="ps_pr")
                            nc.tensor.matmul(
                                box[0][:],
                                wpr_t[4 * cc + r][:, 128 * ct:128 * (ct + 1)],
                                attn_tiles[c][r][:], start=(r == 0),
                                stop=(r == 3))
                            if r == 3:
                                ai = ai_p.tile([128, 512], BF16, name="ai")
                                if drain_scalar:
                                    nc.scalar.activation(ai[:], box[0][:],
                                                         AF.Copy)
                                else:
                                    nc.vector.tensor_copy(ai[:], box[0][:])
                                nc.sync.dma_start(out=ri[blk, dct], in_=ai[:])
                        return f
                    for r in range(4):
                        steps.append((512 * PE_NS, mk(r), r == 0))
            return steps

        # ================= attention for chunk c ===========================
        def att_chunk(c, filler):
            """Pipelined attention units with filler weave.  filler is a
            list of (pe_cost, fn, chain_start); consumed front-to-back."""
            nt = 4 * (c + 1)
            units = [(krt, t) for krt in range(4) for t in range(nt)]
            qtb = qtb_tiles[c]
            pe_ns = 0.0
            sc_ns = 0.0
            fq = list(filler)
            fi = [0]
            cur = [0]
            state = {}      # krt -> (pa0, pa1)
            P_of = {}       # unit -> (P, pt, qo)
            post = []       # deferred (due_idx, fn) — rb matmuls

            def service_post(force=False):
                nonlocal pe_ns
                while post and (force or post[0][0] <= cur[0]):
                    post.pop(0)[1]()
                    pe_ns += 1024 * PE_NS

            def fill():
                nonlocal pe_ns
                while fi[0] < len(fq) and pe_ns < sc_ns:
                    cost, fn, st = fq[fi[0]]
                    if st:
                        service_post()
                    fi[0] += 1
                    fn()
                    pe_ns += cost
                if fi[0] >= len(fq):
                    service_post()

            def emit_S(u):
                nonlocal pe_ns, sc_ns
                krt, t = u
                j = t - 4 * c
                qo = 128 * j if j >= 0 else 0
                P = psS.tile([128, 1024], F32, tag="ps_s", name="P")
                for h in range(2):
                    o = 512 * h
                    nc.tensor.matmul(
                        P[:, o + qo:o + 512],
                        kt_v[64 * h:64 * (h + 1), krt, 128 * t:128 * (t + 1)],
                        qtb[krt][64 * h:64 * (h + 1), qo:512],
                        start=True, stop=(j < 0))
                    if j >= 0:
                        nc.tensor.matmul(
                            P[:, o + qo:o + qo + 128], trin_sb[:],
                            eye_sb[:], start=False, stop=True)
                pe_ns += 2 * (512 - qo) * PE_NS + (256 * PE_NS if j >= 0
                                                  else 0)
                pt = pt_p.tile([128, 1024], BF16, name="pt")
                pv = P[:].rearrange("p (z q) -> p z q", q=512)
                tv = pt[:].rearrange("p (z q) -> p z q", q=512)
                nc.scalar.activation(tv[:, :, qo:], pv[:, :, qo:], AF.Exp,
                                     bias=lna_sb[:, t:t + 1], scale=0.125)
                sc_ns += 2 * (512 - qo) * EXP_NS + 250
                P_of[u] = (P, pt, qo)

            def emit_A(u):
                nonlocal pe_ns
                krt, t = u
                P, pt, qo = P_of.pop(u)
                if t == 0:
                    pa0 = psA.tile([128, 512], F32, tag="psa", name="pa0")
                    pa1 = psA.tile([128, 512], F32, tag="psa", name="pa1")
                    state[krt] = (pa0, pa1)
                pa0, pa1 = state[krt]
                h0, h1 = 2 * krt, 2 * krt + 1
                nc.tensor.matmul(pa0[0:65, qo:], v_v[:, t, 65 * h0:
                                                      65 * h0 + 65],
                                 pt[:, qo:512], start=(t == 0),
                                 stop=(t == nt - 1))
                nc.tensor.matmul(pa1[0:65, qo:], v_v[:, t, 65 * h1:
                                                      65 * h1 + 65],
                                 pt[:, 512 + qo:1024], start=(t == 0),
                                 stop=(t == nt - 1))
                pe_ns += 2 * (512 - qo) * PE_NS

            def krt_end(krt, idx):
                """attnTb copies + den reciprocal; rb matmul deferred."""
                nonlocal sc_ns
                pa0, pa1 = state.pop(krt)
                at = attnTb_p.tile([128, 512], BF16, name="at")
                attn_tiles[c][krt] = at
                nc.scalar.activation(at[0:64, :], pa0[0:64, :], AF.Copy)
                t64 = t64_p.tile([64, 512], BF16, name="t64")
                nc.scalar.activation(t64[:], pa1[0:64, :], AF.Copy)
                nc.sync.dma_start(out=at[64:128, :], in_=t64[:])
                sc_ns += 2 * 512 * EXP_NS + 500
                dd0 = dd_p.tile([65, 512], F32, tag="dd", name="dd0")
                nc.vector.tensor_copy(dd0[64:65, :], pa0[64:65, :])
                dd1 = dd_p.tile([65, 512], F32, tag="dd", name="dd1")
                nc.vector.tensor_copy(dd1[64:65, :], pa1[64:65, :])
                den2 = ddb_p.tile([2, 512], F32, tag="den2", name="den2")
                nc.sync.dma_start(out=den2[0:1, :], in_=dd0[64:65, :])
                nc.sync.dma_start(out=den2[1:2, :], in_=dd1[64:65, :])
                rec2 = ddb_p.tile([2, 512], F32, tag="rec2", name="rec2")
                nc.vector.reciprocal_approx_fast(rec2[:], den2[:])
                rcb = ddb_p.tile([2, 512], BF16, tag="rcb", name="rcb")
                nc.vector.tensor_copy(rcb[:], rec2[:])

                def rb_fn():
                    rb = psM.tile([128, 512], F32, tag="mm", name="rb")
                    nc.tensor.matmul(rb[:], sel2b_sb[:], rcb[:],
                                     start=True, stop=True)
                    nc.vector.tensor_mul(at[:], at[:], rb[:])
                post.append((idx + 4, rb_fn))

            def fill_one():
                nonlocal pe_ns
                if fi[0] < len(fq):
                    cost, fn, st = fq[fi[0]]
                    if st:
                        service_post()
                    fi[0] += 1
                    fn()
                    pe_ns += cost

            for idx, u in enumerate(units):
                cur[0] = idx
                emit_S(u)
                if idx > 0:
                    fill_one()
                    fill_one()
                    fill_one()
                    prev = units[idx - 1]
                    emit_A(prev)
                    if prev[1] == nt - 1:
                        krt_end(prev[0], idx)
                fill()
            cur[0] = len(units)
            emit_A(units[-1])
            krt_end(3, len(units))
            # close any open filler chain, then flush deferred rbs + rest
            while fi[0] < len(fq) and not fq[fi[0]][2]:
                cost, fn, st = fq[fi[0]]
                fi[0] += 1
                fn()
            service_post(force=True)
            while fi[0] < len(fq):
                cost, fn, st = fq[fi[0]]
                fi[0] += 1
                fn()

        # ---- shared LN helpers ----
        def ln_stat_begin(sp):
            if sp is psS:
                ps_sumA = psS.tile([1, 512], F32, tag="ps_s", name="ps_sumA")
                ps_sumB = psS.tile([1, 512], F32, tag="ps_s", name="ps_sumB")
            else:
                ps_sumA = psA.tile([1, 512], F32, tag="psa", name="ps_sumA")
                ps_sumB = psA.tile([1, 512], F32, tag="psa", name="ps_sumB")
            return ps_sumA, ps_sumB

        def ln_stat_tile(ps_sumA, ps_sumB, src, i):
            tb = cast_p.tile([128, 512], BF16, name="tb")
            nc.vector.tensor_copy(tb[:], src[:])
            nc.tensor.matmul(ps_sumA[:], ones_col_b[:], tb[:],
                             start=(i == 0), stop=(i == DT - 1))
            sqt = sq_p.tile([128, 512], BF16, name="sqt")
            nc.scalar.activation(sqt[:], src[:], AF.Square)
            nc.tensor.matmul(ps_sumB[:], ones_col_b[:], sqt[:],
                             start=(i == 0), stop=(i == DT - 1))

        def ln_stat_finish(ps_sumA, ps_sumB):
            st = strow_p.tile([1, 3 * 512], F32, tag="st", name="st")
            sA, sB2, sC = st[:, 0:512], st[:, 512:1024], st[:, 1024:1536]
            nc.scalar.activation(sA, ps_sumA[:], AF.Copy, scale=1.0 / D)  # u
            nc.scalar.activation(sB2, ps_sumB[:], AF.Identity,
                                 bias=eps_sb[:], scale=1.0 / D)   # msq+eps
            nc.scalar.activation(sC, sA, AF.Square)               # u^2
            nc.vector.tensor_sub(sB2, sB2, sC)                    # var
            nc.vector.reciprocal_approx_fast(sC, sB2)             # 1/var
            nc.scalar.activation(sB2, sC, AF.Sqrt)                # rstd
            nc.vector.tensor_mul(sC, sA, sB2)                     # u*rstd
            rstd_ps = psA.tile([128, 512], F32, tag="psa", name="rstd_ps")
            nc.tensor.matmul(rstd_ps[:], ones_row_f[:], sB2,
                             start=True, stop=True)
            urstd_ps = psA.tile([128, 512], F32, tag="psa", name="urstd_ps")
            nc.tensor.matmul(urstd_ps[:], ones_row_f[:], sC,
                             start=True, stop=True)
            urstd_sb = ubc_p.tile([128, 512], F32, name="urstd_sb")
            nc.vector.tensor_copy(urstd_sb[:], urstd_ps[:])
            return rstd_ps, urstd_sb

        def ln_stats(src_t, sp):
            pA, pB = ln_stat_begin(sp)
            for i in range(DT):
                ln_stat_tile(pA, pB, src_t[i], i)
            return ln_stat_finish(pA, pB)

        def layernorm_to_bf16(src_t, g_sb, b_sb, use_gb, sp):
            rstd_ps, urstd_ps = ln_stats(src_t, sp)
            out_t = []
            for i in range(DT):
                tmpn = tmpn_p.tile([128, 512], F32, name="tmpn")
                nc.vector.tensor_mul(tmpn[:], src_t[i][:], rstd_ps[:])
                nb = nTb_p.tile([128, 512], BF16, tag="nTb", name="nb")
                nc.vector.tensor_sub(nb[:], tmpn[:], urstd_ps[:])
                if use_gb:
                    nc.vector.tensor_scalar(nb[:], nb[:], g_sb[:, i:i + 1],
                                            b_sb[:, i:i + 1], OP.mult, OP.add)
                out_t.append(nb)
            return out_t

        # ============== B: full-width MLP for owned chunk k ================
        def B_res(k):
            """Residual loads + adds (no PE).  Returns t1 tiles."""
            ro = rs_out[k][:].rearrange("(i p) q -> i p q", p=128)
            t1_t = []
            for i in range(DT):
                rob = rob_p.tile([128, 512], BF16, name="rob")
                nc.gpsimd.dma_start(out=rob[:], in_=ro[i])
                xf2 = xf2_p.tile([128, 512], F32, name="xf2")
                nc.gpsimd.dma_start(out=xf2[:], in_=xo[DT * k + i])
                t1 = t1_p.tile([128, 512], F32, name="t1")
                nc.gpsimd.tensor_add(t1[:], rob[:], xf2[:])
                if use_projb:
                    nc.gpsimd.tensor_scalar_add(t1[:], t1[:],
                                                projb_sb[:, i:i + 1])
                t1_t.append(t1)
            return t1_t

        def load_wf(fg):
            pair = []
            for half in range(2):
                t_ = wf_p.tile([128, 4 * 512], BF16, name="wfh")
                nc.sync.dma_start(out=t_[:], in_=wfc[fg, half])
                pair.append(t_)
            return pair

        def B(k, nTb_t, wf_pre=None):
            yield

            # ---- fc + gelu (weights double-buffered one group ahead) ----
            gT_t = []
            pre = list(wf_pre) if wf_pre else [load_wf(0), load_wf(1)]
            for fg in range(8):
                wfh = pre.pop(0)
                if fg + 2 < 8:
                    pre.append(load_wf(fg + 2))
                for ct in range(4):
                    f = 4 * fg + ct
                    ps = psM.tile([128, 512], F32, tag="mm", name="ps_fc")
                    for d in range(DT):
                        w = wfh[d // 4]
                        dd = d % 4
                        nc.tensor.matmul(
                            ps[:],
                            w[:, 512 * dd + 128 * ct:512 * dd + 128 * (ct + 1)],
                            nTb_t[d][:], start=(d == 0), stop=(d == DT - 1))
                    gt = gT_p.tile([128, 512], BF16, name="gt")
                    nc.scalar.activation(gt[:], ps[:], AF.Gelu_apprx_tanh,
                                         bias=fcb_sb[:, f:f + 1])
                    gT_t.append(gt)
                yield

            # ---- cproj with LN2 stats incrementally ----
            dumm = strow_p.tile([1, 3 * 512], F32, tag="st", name="st")
            nc.scalar.activation(dumm[:, 0:1], gT_t[-1][0:1, 0:1],
                                 AF.Sqrt)
            mar_t = []
            pA2, pB2 = ln_stat_begin(psS)
            for p4 in range(4):
                wcq = []
                for qtr in range(4):
                    t_ = wc_p.tile([128, 8 * 256], BF16, name="wcq")
                    nc.sync.dma_start(out=t_[:], in_=wcp[p4, qtr])
                    wcq.append(t_)
                for ci in range(2):
                    dct = 2 * p4 + ci
                    ps = psM.tile([128, 512], F32, tag="mm", name="ps_cp")
                    for f in range(FT):
                        w = wcq[f // 8]
                        fi_ = f % 8
                        nc.tensor.matmul(
                            ps[:],
                            w[:, 256 * fi_ + 128 * ci:256 * fi_ + 128 * (ci + 1)],
                            gT_t[f][:], start=(f == 0), stop=(f == FT - 1))
                    m2 = t1_p.tile([128, 512], F32, name="t1")
                    nc.vector.tensor_add(m2[:], ps[:], nTb_t[dct][:])
                    if use_cprojb:
                        nc.vector.tensor_scalar_add(
                            m2[:], m2[:], cprojb_sb[:, dct:dct + 1])
                    mar_t.append(m2)
                    ln_stat_tile(pA2, pB2, m2, dct)
                yield

            # ---- LN2 -> output ----
            rstd_ps, urstd_ps = ln_stat_finish(pA2, pB2)
            for i in range(DT):
                eng = nc.vector
                ht = hT_p.tile([128, 512], F32, tag="hT", name="ht")
                eng.tensor_mul(ht[:], mar_t[i][:], rstd_ps[:])
                eng.tensor_sub(ht[:], ht[:], urstd_ps[:])
                if use_g2b2:
                    eng.tensor_scalar(ht[:], ht[:], g2_sb[:, i:i + 1],
                                      b2_sb[:, i:i + 1],
                                      OP.mult, OP.add)
                nc.sync.dma_start(out=out[DT * k + i], in_=ht[:])
            yield

        def run(g):
            for _ in g:
                pass

        def trigger_rs(j):
            nc.gpsimd.collective_compute(
                "ReduceScatter", OP.add, replica_groups=PAIRS,
                ins=[rs_in[j][:].opt()], outs=[rs_out[j][:].opt()])

        # =========================== schedule ==============================
        # prologue: QKV(0) as a solid block
        for cost, fn, st in qkv_steps(0):
            fn()

        att_chunk(0, qkv_steps(1))
        att_chunk(1, qkv_steps(2) + proj_steps(0))
        # proj(1) right after att(1) so RS0 can fire early
        for cost, fn, st in proj_steps(1):
            fn()
        trigger_rs(0)
        att_chunk(2, qkv_steps(3))
        t1_b0_box = [None]

        def b0_res_steps():
            def f():
                t1_b0_box[0] = B_res(0)
            return [(0, f, True)]
        att_chunk(3, proj_steps(2) + b0_res_steps())
        # LN1(B0) stats before proj(3): scalar finish chain, stat->bcast
        # latency and the normalize DVE passes all hide under proj(3).
        # tile_wait_until keeps the scheduler from hoisting these RS-gated
        # ops into earlier queue slots where they would block the engine.
        with tc.tile_wait_until(0.255):
            pA1, pB1 = ln_stat_begin(psS)
            for i in range(DT):
                ln_stat_tile(pA1, pB1, t1_b0_box[0][i], i)
        p3 = proj_steps(3, drain_scalar=True)
        for cost, fn, st in p3[:12]:
            fn()
        with tc.tile_wait_until(0.258):
            rstd1, urstd1 = ln_stat_finish(pA1, pB1)
        nTb0 = []

        def norm0(i):
            with tc.tile_wait_until(0.26):
                eng = nc.vector
                tmpn = tmpn_p.tile([128, 512], F32, name="tmpn")
                eng.tensor_mul(tmpn[:], t1_b0_box[0][i][:], rstd1[:])
                nb = nTb_p.tile([128, 512], BF16, tag="nTb", name="nb")
                eng.tensor_sub(nb[:], tmpn[:], urstd1[:])
                if use_g1b1:
                    eng.tensor_scalar(nb[:], nb[:], g1_sb[:, i:i + 1],
                                      b1_sb[:, i:i + 1],
                                      OP.mult, OP.add)
                nTb0.append(nb)
        ni = 0
        for si, (cost, fn, st) in enumerate(p3[12:]):
            fn()
            if si % 2 == 1 and ni < DT:
                norm0(ni)
                ni += 1
        while ni < DT:
            norm0(ni)
            ni += 1
        # release attention weights, open MLP weight pools, then prefetch
        # fc(B0) weights ahead of the collective's DMA traffic
        actx.close()
        wf_p = pool("wf_p", 5)          # fc weight half-groups [128, 2048]
        gT_p = pool("gT_p", 32)         # gelu outputs bf16
        wc_p = pool("wc_p", 3)          # cproj weight quarter [128, 2048]
        wf_pre0 = [load_wf(0), load_wf(1)]
        trigger_rs(1)

        b0 = B(0, nTb0, wf_pre0)
        for _ in range(10):
            next(b0)      # (noop), fc fg0..7, cproj p4-0
        with tc.tile_wait_until(0.41):
            nTb1 = layernorm_to_bf16(B_res(1), g1_sb, b1_sb, use_g1b1, psA)
        b1 = B(1, nTb1)
        next(b1)          # noop yield
        for _ in range(3):
            next(b0)      # cproj p4 1-3
        for _ in range(2):
            next(b1)      # fc fg0, fg1 — cover b0's LN2 finish
        run(b0)           # LN2(B0) finish + normalize + out
        run(b1)           # fc rest, cproj, LN2

    nc.compile()
    return nc


_cache = {}


def _get_program(flags):
    if flags not in _cache:
        _cache[flags] = _build(*flags)
    return _cache[flags]


def _pack(a, rows, cols):
    """[R, C] -> [R//rows * C//cols, rows, cols], row-tile-major."""
    R, C = a.shape
    return np.ascontiguousarray(
        a.reshape(R // rows, rows, C // cols, cols).transpose(0, 2, 1, 3)
        .reshape(-1, rows, cols))


def _swap(p, n_rt, n_ct):
    """_pack gives (row-tile, col-tile) order; swap to (col, row)."""
    t = p.reshape(n_rt, n_ct, p.shape[1], p.shape[2])
    return np.ascontiguousarray(
        t.transpose(1, 0, 2, 3).reshape(-1, p.shape[1], p.shape[2]))


def _rowpack(tiles):
    """[N, 128, C] tile list -> [128, N*C] contiguous-row layout."""
    n, p, c = tiles.shape
    return np.ascontiguousarray(tiles.transpose(1, 0, 2).reshape(p, n * c))


def _prepare_inputs(inputs):
    x = np.asarray(inputs["x"], np.float32)
    weight = float(np.asarray(inputs["weight"]).reshape(-1)[0])
    linear_w = np.asarray(inputs["linear_w"], np.float32)
    linear_b = np.asarray(inputs["linear_b"], np.float32)
    proj_w = np.asarray(inputs["proj_w"], np.float32)
    proj_b = np.asarray(inputs["proj_b"], np.float32)
    ln1_g = np.asarray(inputs["ln1_g"], np.float32)
    ln1_b = np.asarray(inputs["ln1_b"], np.float32)
    fc_w = np.asarray(inputs["fc_w"], np.float32)
    fc_b = np.asarray(inputs["fc_b"], np.float32)
    cproj_w = np.asarray(inputs["cproj_w"], np.float32)
    cproj_b = np.asarray(inputs["cproj_b"], np.float32)
    ln2_g = np.asarray(inputs["ln2_g"], np.float32)
    ln2_b = np.asarray(inputs["ln2_b"], np.float32)
    idx = np.asarray(inputs["idx"]).astype(np.int64).reshape(-1)
    forcing = bool(np.asarray(inputs["attn_forcing"]).reshape(-1)[0])

    flags = (
        bool(linear_b[:2048].any()),      # use_bqk
        bool(linear_b[2048:].any()),      # use_bv
        bool(proj_b.any()),
        bool(cproj_b.any()),
        bool((ln1_g != 1).any() or ln1_b.any()),
        bool((ln2_g != 1).any() or ln2_b.any()),
    )

    if forcing:
        lnw = float(np.log(weight)) if weight >= 1e-37 else -1e9
        pos = np.arange(S)
        lna_all = np.where(pos[None, :] >= idx[:, None], lnw, 0.0) \
            .astype(np.float32)
    else:
        lna_all = np.zeros((B, S), np.float32)

    # strict upper triangle gets -1e9 (causal mask via matmul accumulate):
    # out[k, qq] += trin[qq, k] must be -1e9 when k > qq.
    trin_np = np.where(np.arange(128)[None, :] > np.arange(128)[:, None],
                       np.float32(-1e9), np.float32(0.0)).astype(BF)
    eye_np = np.eye(128, dtype=np.float32).astype(BF)
    sel2_np = np.zeros((1, 256), np.float32)
    sel2_np[0, 0:64] = 1.0       # even-head recip -> partitions 0..63
    sel2_np[0, 128 + 64:] = 1.0  # odd-head recip -> partitions 64..127
    sel2_np = sel2_np.astype(BF)

    # ---- global (all-core) MLP weights ----
    wfc_p = _swap(_pack(fc_w.astype(BF), 128, 512), DT, 8)   # (fg, d)
    wfc_g = np.ascontiguousarray(
        wfc_p.reshape(8, 2, 4, 128, 512).transpose(0, 1, 3, 2, 4)
        .reshape(8, 2, 128, 4 * 512))
    wcp_p = _swap(_pack(cproj_w.astype(BF), 128, 256), FT, 4)  # (p4, f)
    wcp_g = np.ascontiguousarray(
        wcp_p.reshape(4, 4, 8, 128, 256).transpose(0, 1, 3, 2, 4)
        .reshape(4, 4, 128, 8 * 256))

    in_maps = []
    for core in range(N_CORES):
        b, r = core // 2, core % 2
        q_cols = slice(512 * r, 512 * (r + 1))
        k_cols = slice(1024 + 512 * r, 1024 + 512 * (r + 1))
        v_cols = slice(2048 + 512 * r, 2048 + 512 * (r + 1))
        xT = np.ascontiguousarray(x[b].T)                       # [D, S]
        wqk_full = np.concatenate([linear_w[:, q_cols], linear_w[:, k_cols]],
                                  axis=1)                       # [D, 1024]

        xq_t = _pack(xT, 128, 512)                  # (d, c): index d*NCH+c
        xq_dc = xq_t.reshape(DT, NCH, 128, 512)
        xqb = np.ascontiguousarray(
            xq_dc.transpose(1, 0, 2, 3).reshape(NCH, 2, 4, 128, 512)
            .transpose(0, 1, 3, 2, 4).reshape(NCH, 2, 128, 4 * 512)
        ).astype(BF)
        own = [r, 2 + r]
        xo_np = np.ascontiguousarray(
            xq_dc[:, own].transpose(1, 0, 2, 3).reshape(2 * DT, 128, 512))

        in_maps.append({
            "xqb": xqb,
            "xo": xo_np,
            "wqk": _rowpack(_swap(_pack(wqk_full.astype(BF), 128, 512),
                                  8, 2)),
            "bqk": np.ascontiguousarray(
                np.concatenate([linear_b[q_cols], linear_b[k_cols]])),
            "wv": _rowpack(_pack(linear_w[:, v_cols].astype(BF), 128, 512)),
            "bv": np.ascontiguousarray(linear_b[v_cols]).astype(BF),
            "wproj": _rowpack(_swap(_pack(proj_w[512 * r:512 * (r + 1), :]
                                          .astype(BF), 128, 512), 4, 2)),
            "projb": proj_b,
            "wfc": wfc_g, "fcb": fc_b,
            "wcp": wcp_g, "cprojb": cproj_b,
            "g1": ln1_g, "b1": ln1_b, "g2": ln2_g, "b2": ln2_b,
            "lna": lna_all[b],
            "trin": trin_np,
            "eye": eye_np,
            "sel2": sel2_np,
        })
    return flags, in_maps


def _unpack_out(o_pair):
    """o_pair: [out_core0, out_core1] each [2*DT, 128, 512] -> [S, D]."""
    hT = np.empty((D, S), np.float32)
    for r in range(2):
        o = o_pair[r].reshape(2, DT, 128, 512)
        for k, c in enumerate([r, 2 + r]):
            hT[:, CH * c:CH * (c + 1)] = o[k].reshape(D, 512)
    return hT.T


def _run(inputs, trace=False):
    flags, in_maps = _prepare_inputs(inputs)
    nc = _get_program(flags)
    r = run_bass_kernel_spmd(nc, in_maps, core_ids=list(range(N_CORES)),
                             trace=trace)
    outs = np.stack(
        [_unpack_out([r.results[2 * b]["out"], r.results[2 * b + 1]["out"]])
         for b in range(B)], axis=0).astype(np.float32)
    return outs, r


def kernel(**inputs) -> np.ndarray:
    outs, _ = _run(inputs, trace=False)
    return outs


# revision 58
# speedup vs baseline: 1.0254x; 1.0254x over previous
"""Dense transformer block on 8 TRN2 NeuronCores.

Sharding: data-parallel over batch (4 pairs of cores). Within each pair:
  - Attention is Megatron head-parallel (8 heads per core, all tokens).
  - Post-attention (residual+LN1+MLP+LN2) is chunk-parallel: partial sums
    of the attention projection for two 512-token chunks are combined with
    one pairwise ReduceScatter per chunk-pair; core r owns chunks {r, 2+r}
    and runs the MLP full-width locally (no second collective).

Device schedule is built around keeping the PE (tensor engine)
continuously busy so it stays at its max p-state clock:
  - Scores for both heads of a (krt, t) unit land in ONE 2-bank PSUM tile
    [128, 1024] and are exponentiated by a single wide ACT instruction.
  - The causal mask is applied by a tiny extra matmul (tri_neg @ I128)
    accumulated into the score PSUM *before* the exp, so there is no
    vector-engine mask pass between exp and attn@V.
  - The PE stream is software-pipelined one unit ahead (S(u+1) issues
    before A(u)), and QKV matmuls of chunk c+1 plus the projection of
    chunk c are woven into attention of chunk c as filler so the PE never
    waits on the Scalar engine's exp stream.
  - Softmax denominators ride attn@V as an extra ones-column of V; the
    two den rows are gathered to partitions 0..1 with a tiny DMA,
    reciprocated with the fast DVE approx, and broadcast across
    partitions with one sel2 @ rec matmul whose emission is deferred a
    few units so the PE stream never waits on the den chain.
  - RS-gated work (residual loads/adds after each ReduceScatter) runs on
    the otherwise idle gpsimd queue, and tile_wait_until hints keep the
    scheduler from hoisting its consumers into engine-queue slots where
    they would head-of-line block attention.
"""

import numpy as np
import ml_dtypes

import concourse.bacc as bacc
import concourse.mybir as mybir
import concourse.tile as tile
from concourse.bass_utils import run_bass_kernel_spmd

F32 = mybir.dt.float32
BF16 = mybir.dt.bfloat16
AF = mybir.ActivationFunctionType
OP = mybir.AluOpType

B, S, D, H, HD, FF = 4, 2048, 1024, 16, 64, 4096
N_CORES = 8
PAIRS = [[0, 1], [2, 3], [4, 5], [6, 7]]
CH = 512                 # tokens per chunk
NCH = S // CH            # 4
DT = D // 128            # 8 d-tiles
FT = FF // 128           # 32 f-tiles
KT = S // 128            # 16 kpos tiles
EPS = 1e-5
BF = ml_dtypes.bfloat16

PE_NS = 0.42             # ns per output column at max clock
EXP_NS = 0.833           # scalar ns per column


def _build(use_bqk, use_bv, use_projb, use_cprojb, use_g1b1, use_g2b2):
    nc = bacc.Bacc("TRN2", target_bir_lowering=False, debug=False,
                   enable_asserts=True, num_devices=N_CORES)

    # ---- DRAM inputs (tile-packed on host) ----
    xqb = nc.dram_tensor("xqb", [NCH, 2, 128, 4 * 512], BF16,
                         kind="ExternalInput")          # bf16 x^T (c, half)
    xo = nc.dram_tensor("xo", [2 * DT, 128, 512], F32,
                        kind="ExternalInput")           # f32 x^T own chunks
    wqk = nc.dram_tensor("wqk", [128, 16 * 512], BF16, kind="ExternalInput")
    bqk = nc.dram_tensor("bqk", [1024], F32, kind="ExternalInput")
    wv = nc.dram_tensor("wv", [128, 8 * 512], BF16, kind="ExternalInput")
    bv = nc.dram_tensor("bv", [512], BF16, kind="ExternalInput")
    wproj = nc.dram_tensor("wproj", [128, 8 * 512], BF16,
                           kind="ExternalInput")
    projb = nc.dram_tensor("projb", [D], F32, kind="ExternalInput")
    wfc = nc.dram_tensor("wfc", [8, 2, 128, 4 * 512], BF16,
                         kind="ExternalInput")          # (fg, half) x (d,q)
    fcb = nc.dram_tensor("fcb", [FF], F32, kind="ExternalInput")
    wcp = nc.dram_tensor("wcp", [4, 4, 128, 8 * 256], BF16,
                         kind="ExternalInput")          # (p4, qtr) x (f,q)
    cprojb = nc.dram_tensor("cprojb", [D], F32, kind="ExternalInput")
    g1 = nc.dram_tensor("g1", [D], F32, kind="ExternalInput")
    b1 = nc.dram_tensor("b1", [D], F32, kind="ExternalInput")
    g2 = nc.dram_tensor("g2", [D], F32, kind="ExternalInput")
    b2 = nc.dram_tensor("b2", [D], F32, kind="ExternalInput")
    lna = nc.dram_tensor("lna", [S], F32, kind="ExternalInput")
    trin = nc.dram_tensor("trin", [128, 128], BF16, kind="ExternalInput")
    eye = nc.dram_tensor("eye", [128, 128], BF16, kind="ExternalInput")
    sel2 = nc.dram_tensor("sel2", [1, 256], BF16, kind="ExternalInput")
    # output: own chunks (k, i) tiles; host reassembles
    out = nc.dram_tensor("out", [2 * DT, 128, 512], F32,
                         kind="ExternalOutput")

    from contextlib import ExitStack
    with tile.TileContext(nc) as tc, ExitStack() as ctx:
        def pool(name, bufs, space="SBUF"):
            return ctx.enter_context(
                tc.tile_pool(name=name, bufs=bufs, space=space))

        const = pool("const", 1)
        xb_p = pool("xb_p", 2)          # bf16 x half-chunks [128, 2048]
        qTb_p = pool("qTb_p", 6)
        pt_p = pool("pt_p", 4)          # exp outputs [128, 1024] bf16
        attnTb_p = pool("attnTb_p", 8)
        dd_p = pool("dd_p", 2)          # den recip rows f32
        ddb_p = pool("ddb_p", 2)        # den recip rows bf16
        t64_p = pool("t64_p", 2)        # odd-head attn staging for DMA move
        ai_p = pool("ai_p", 2)          # proj partial bf16 tiles
        rob_p = pool("rob_p", 2)        # bf16 rs_out staging
        t1_p = pool("t1_p", 10)         # B: residual tiles f32 (t1 AND n+m)
        xf2_p = pool("xf2_p", 2)
        cast_p = pool("cast_p", 1)      # LN bf16 casts
        sq_p = pool("sq_p", 1)
        strow_p = pool("strow_p", 1)
        nTb_p = pool("nTb_p", 8)        # bf16 n tiles (fc rhs + s3 residual)
        tmpn_p = pool("tmpn_p", 1)
        hT_p = pool("hT_p", 3)
        ubc_p = pool("ubc_p", 1)        # u*rstd SBUF copy
        psS = pool("psS", 2, "PSUM")    # [128,1024] score pair tiles
        psA = pool("psA", 2, "PSUM")    # [128,512] av accumulators + bcast
        psM = pool("psM", 2, "PSUM")    # [128,512] general matmul tiles
        dram = pool("dram", 2, "DRAM")

        # ---- constants ----
        kt_sb = const.tile([128, 4 * S], BF16, name="kt_sb")
        kt_v = kt_sb[:].rearrange("p (r q) -> p r q", q=S)
        v_sb = const.tile([128, KT * 520], BF16, name="v_sb")
        v_v = v_sb[:].rearrange("p (t e) -> p t e", e=520)
        # ones column of V (den rides attn@V as the 65th row), set once
        v_h = v_sb[:].rearrange("p (t h e) -> p t h e", h=8, e=65)
        nc.vector.memset(v_h[:, :, :, 64:65], 1.0)

        # prefetch chunk 0's x before the big weight DMAs
        xh0 = []
        for half in range(2):
            t_ = xb_p.tile([128, 4 * 512], BF16, name="xh0")
            nc.sync.dma_start(out=t_[:], in_=xqb[0, half])
            xh0.append(t_)

        # ---- resident attention weights (pool closed before B phase so
        # the MLP weight pools can reuse its SBUF space) ----
        actx = ExitStack()
        wres = actx.enter_context(
            tc.tile_pool(name="wres", bufs=1, space="SBUF"))
        wqk_sb = wres.tile([128, 16 * 512], BF16, name="wqk_sb")
        nc.sync.dma_start(out=wqk_sb[:, 0:8 * 512], in_=wqk[:, 0:8 * 512])
        nc.scalar.dma_start(out=wqk_sb[:, 8 * 512:], in_=wqk[:, 8 * 512:])
        wqk_t = [wqk_sb[:, 512 * i:512 * (i + 1)] for i in range(16)]
        wv_sb = wres.tile([128, 8 * 512], BF16, name="wv_sb")
        nc.scalar.dma_start(out=wv_sb[:], in_=wv[:])
        wv_t = [wv_sb[:, 512 * i:512 * (i + 1)] for i in range(8)]
        wpr_sb = wres.tile([128, 8 * 512], BF16, name="wpr_sb")
        nc.scalar.dma_start(out=wpr_sb[:], in_=wproj[:])
        wpr_t = [wpr_sb[:, 512 * i:512 * (i + 1)] for i in range(8)]

        trin_sb = const.tile([128, 128], BF16, name="trin_sb")
        nc.scalar.dma_start(out=trin_sb[:], in_=trin[:])
        eye_sb = const.tile([128, 128], BF16, name="eye_sb")
        nc.scalar.dma_start(out=eye_sb[:], in_=eye[:])
        sel2b_sb = const.tile([2, 128], BF16, name="sel2b_sb")
        nc.scalar.dma_start(out=sel2b_sb[:],
                            in_=sel2.rearrange("o (a b) -> (o a) b", a=2))
        lna_sb = const.tile([128, KT], F32, name="lna_sb")
        nc.scalar.dma_start(out=lna_sb[:],
                            in_=lna.rearrange("(t p) -> p t", p=128))
        ones_col_b = const.tile([128, 1], BF16, name="ones_col_b")
        nc.vector.memset(ones_col_b[:], 1.0)
        ones_row_f = const.tile([1, 128], F32, name="ones_row_f")
        nc.vector.memset(ones_row_f[:], 1.0)
        eps_sb = const.tile([1, 1], F32, name="eps_sb")
        nc.vector.memset(eps_sb[:], EPS)
        fcb_sb = const.tile([128, FT], F32, name="fcb_sb")
        nc.scalar.dma_start(out=fcb_sb[:],
                            in_=fcb.rearrange("(i p) -> p i", p=128))

        def vec8(name, t):
            sb = const.tile([128, DT], F32, name=name)
            nc.scalar.dma_start(out=sb[:],
                                in_=t.rearrange("(i p) -> p i", p=128))
            return sb

        bqk_sb = vec8("bqk_sb", bqk) if use_bqk else None
        projb_sb = vec8("projb_sb", projb) if use_projb else None
        cprojb_sb = vec8("cprojb_sb", cprojb) if use_cprojb else None
        g1_sb = vec8("g1_sb", g1) if use_g1b1 else None
        b1_sb = vec8("b1_sb", b1) if use_g1b1 else None
        g2_sb = vec8("g2_sb", g2) if use_g2b2 else None
        b2_sb = vec8("b2_sb", b2) if use_g2b2 else None
        if use_bv:
            ones_row_b = const.tile([1, 128], BF16, name="ones_row_b")
            nc.vector.memset(ones_row_b[:], 1.0)
            bv_sb = const.tile([1, 512], BF16, name="bv_sb")
            nc.sync.dma_start(out=bv_sb[:],
                              in_=bv.rearrange("(o q) -> o q", o=1))

        # ---- ReduceScatter buffers (bf16 payload halves the wire) ----
        rs_in = [dram.tile([2 * D, 512], BF16, tag=f"rsi{j}",
                           name=f"rs_in{j}") for j in range(2)]
        rs_out = [dram.tile([D, 512], BF16, tag=f"rso{j}",
                            name=f"rs_out{j}") for j in range(2)]

        qtb_tiles = {0: [None] * 4, 1: [None] * 4, 2: [None] * 4,
                     3: [None] * 4}
        attn_tiles = {c: [None] * 4 for c in range(4)}

        # ================= QKV filler steps for chunk c ====================
        # Steps are (pe_cost_ns, closure, chain_start).  chain_start marks
        # safe points where a deferred rb matmul may allocate from the
        # shared psM ring without colliding with an open accumulation chain.
        def qkv_steps(c):
            tok = slice(CH * c, CH * (c + 1))
            steps = []
            if c == 0:
                xh = xh0
            else:
                xh = [None, None]

                def load(half):
                    def f():
                        t_ = xb_p.tile([128, 4 * 512], BF16, name="xh")
                        nc.sync.dma_start(out=t_[:], in_=xqb[c, half])
                        xh[half] = t_
                    return f
                steps.append((0, load(0), True))
                steps.append((0, load(1), True))

            def xtb(d):
                return xh[d // 4][:, 512 * (d % 4):512 * (d % 4 + 1)]

            # 8 QK chains (i<4: Q -> qTb, else K -> kt_v)
            for cc in range(2):
                for ct in range(4):
                    i = 4 * cc + ct
                    box = [None]

                    def mk(d, i=i, cc=cc, ct=ct, box=box):
                        def f():
                            if d == 0:
                                box[0] = psM.tile([128, 512], F32, tag="mm",
                                                  name="ps_qk")
                            nc.tensor.matmul(
                                box[0][:],
                                wqk_t[8 * cc + d][:, 128 * ct:128 * (ct + 1)],
                                xtb(d), start=(d == 0), stop=(d == DT - 1))
                            if d == DT - 1:
                                if i < 4:
                                    dt_ = qTb_p.tile([128, 512], BF16,
                                                     name="qTb")
                                    qtb_tiles[c][i] = dt_
                                    dest = dt_[:]
                                else:
                                    dest = kt_v[:, i - 4, tok]
                                if use_bqk:
                                    nc.vector.tensor_scalar_add(
                                        dest, box[0][:], bqk_sb[:, i:i + 1])
                                else:
                                    nc.vector.tensor_copy(dest, box[0][:])
                        return f
                    for d in range(DT):
                        steps.append((512 * PE_NS, mk(d), d == 0))

            # 4 V chains
            for tt in range(4):
                tg = 4 * c + tt
                box = [None]

                def mkv(d, tt=tt, tg=tg, box=box):
                    def f():
                        if d == 0:
                            box[0] = psM.tile([128, 512], F32, tag="mm",
                                              name="ps_v")
                        nc.tensor.matmul(
                            box[0][:], xtb(d)[:, 128 * tt:128 * (tt + 1)],
                            wv_t[d], start=(d == 0),
                            stop=(d == DT - 1 and not use_bv))
                        if d == DT - 1:
                            if use_bv:
                                nc.tensor.matmul(box[0][:], ones_row_b[:],
                                                 bv_sb[:], start=False,
                                                 stop=True)
                            ps_h = box[0][:].rearrange(
                                "p (h e) -> p h e", e=64)
                            nc.vector.tensor_copy(
                                v_h[:, tg, :, 0:64], ps_h[:])
                    return f
                for d in range(DT):
                    steps.append((512 * PE_NS, mkv(d), d == 0))
            return steps

        # ================= proj steps for chunk c ==========================
        def proj_steps(c, drain_scalar=False):
            blk = c % 2
            ri = rs_in[c // 2][:] \
                .rearrange("(k i p) q -> k i p q", k=2, p=128)
            steps = []
            for cc in range(2):
                for ct in range(4):
                    dct = 4 * cc + ct
                    box = [None]

                    def mk(r, cc=cc, ct=ct, dct=dct, box=box):
                        def f():
                            if r == 0:
                                box[0] = psM.tile([128, 512], F32, tag="mm",
                                                  name="ps_pr")
                            nc.tensor.matmul(
                                box[0][:],
                                wpr_t[4 * cc + r][:, 128 * ct:128 * (ct + 1)],
                                attn_tiles[c][r][:], start=(r == 0),
                                stop=(r == 3))
                            if r == 3:
                                ai = ai_p.tile([128, 512], BF16, name="ai")
                                if drain_scalar:
                                    nc.scalar.activation(ai[:], box[0][:],
                                                         AF.Copy)
                                else:
                                    nc.vector.tensor_copy(ai[:], box[0][:])
                                nc.sync.dma_start(out=ri[blk, dct], in_=ai[:])
                        return f
                    for r in range(4):
                        steps.append((512 * PE_NS, mk(r), r == 0))
            return steps

        # ================= attention for chunk c ===========================
        def att_chunk(c, filler):
            """Pipelined attention units with filler weave.  filler is a
            list of (pe_cost, fn, chain_start); consumed front-to-back."""
            nt = 4 * (c + 1)
            units = [(krt, t) for krt in range(4) for t in range(nt)]
            qtb = qtb_tiles[c]
            pe_ns = 0.0
            sc_ns = 0.0
            fq = list(filler)
            fi = [0]
            cur = [0]
            state = {}      # krt -> (pa0, pa1)
            P_of = {}       # unit -> (P, pt, qo)
            post = []       # deferred (due_idx, fn) — rb matmuls

            def service_post(force=False):
                nonlocal pe_ns
                while post and (force or post[0][0] <= cur[0]):
                    post.pop(0)[1]()
                    pe_ns += 1024 * PE_NS

            def fill():
                nonlocal pe_ns
                while fi[0] < len(fq) and pe_ns < sc_ns:
                    cost, fn, st = fq[fi[0]]
                    if st:
                        service_post()
                    fi[0] += 1
                    fn()
                    pe_ns += cost
                if fi[0] >= len(fq):
                    service_post()

            def emit_S(u):
                nonlocal pe_ns, sc_ns
                krt, t = u
                j = t - 4 * c
                qo = 128 * j if j >= 0 else 0
                P = psS.tile([128, 1024], F32, tag="ps_s", name="P")
                for h in range(2):
                    o = 512 * h
                    nc.tensor.matmul(
                        P[:, o + qo:o + 512],
                        kt_v[64 * h:64 * (h + 1), krt, 128 * t:128 * (t + 1)],
                        qtb[krt][64 * h:64 * (h + 1), qo:512],
                        start=True, stop=(j < 0))
                    if j >= 0:
                        nc.tensor.matmul(
                            P[:, o + qo:o + qo + 128], trin_sb[:],
                            eye_sb[:], start=False, stop=True)
                pe_ns += 2 * (512 - qo) * PE_NS + (256 * PE_NS if j >= 0
                                                  else 0)
                pt = pt_p.tile([128, 1024], BF16, name="pt")
                pv = P[:].rearrange("p (z q) -> p z q", q=512)
                tv = pt[:].rearrange("p (z q) -> p z q", q=512)
                nc.scalar.activation(tv[:, :, qo:], pv[:, :, qo:], AF.Exp,
                                     bias=lna_sb[:, t:t + 1], scale=0.125)
                sc_ns += 2 * (512 - qo) * EXP_NS + 250
                P_of[u] = (P, pt, qo)

            def emit_A(u):
                nonlocal pe_ns
                krt, t = u
                P, pt, qo = P_of.pop(u)
                if t == 0:
                    pa0 = psA.tile([128, 512], F32, tag="psa", name="pa0")
                    pa1 = psA.tile([128, 512], F32, tag="psa", name="pa1")
                    state[krt] = (pa0, pa1)
                pa0, pa1 = state[krt]
                h0, h1 = 2 * krt, 2 * krt + 1
                nc.tensor.matmul(pa0[0:65, qo:], v_v[:, t, 65 * h0:
                                                      65 * h0 + 65],
                                 pt[:, qo:512], start=(t == 0),
                                 stop=(t == nt - 1))
                nc.tensor.matmul(pa1[0:65, qo:], v_v[:, t, 65 * h1:
                                                      65 * h1 + 65],
                                 pt[:, 512 + qo:1024], start=(t == 0),
                                 stop=(t == nt - 1))
                pe_ns += 2 * (512 - qo) * PE_NS

            def krt_end(krt, idx):
                """attnTb copies + den reciprocal; rb matmul deferred."""
                nonlocal sc_ns
                pa0, pa1 = state.pop(krt)
                at = attnTb_p.tile([128, 512], BF16, name="at")
                attn_tiles[c][krt] = at
                nc.scalar.activation(at[0:64, :], pa0[0:64, :], AF.Copy)
                t64 = t64_p.tile([64, 512], BF16, name="t64")
                nc.scalar.activation(t64[:], pa1[0:64, :], AF.Copy)
                nc.sync.dma_start(out=at[64:128, :], in_=t64[:])
                sc_ns += 2 * 512 * EXP_NS + 500
                dd0 = dd_p.tile([65, 512], F32, tag="dd", name="dd0")
                nc.vector.tensor_copy(dd0[64:65, :], pa0[64:65, :])
                dd1 = dd_p.tile([65, 512], F32, tag="dd", name="dd1")
                nc.vector.tensor_copy(dd1[64:65, :], pa1[64:65, :])
                den2 = ddb_p.tile([2, 512], F32, tag="den2", name="den2")
                nc.sync.dma_start(out=den2[0:1, :], in_=dd0[64:65, :])
                nc.sync.dma_start(out=den2[1:2, :], in_=dd1[64:65, :])
                rec2 = ddb_p.tile([2, 512], F32, tag="rec2", name="rec2")
                nc.vector.reciprocal_approx_fast(rec2[:], den2[:])
                rcb = ddb_p.tile([2, 512], BF16, tag="rcb", name="rcb")
                nc.vector.tensor_copy(rcb[:], rec2[:])

                def rb_fn():
                    rb = psM.tile([128, 512], F32, tag="mm", name="rb")
                    nc.tensor.matmul(rb[:], sel2b_sb[:], rcb[:],
                                     start=True, stop=True)
                    nc.vector.tensor_mul(at[:], at[:], rb[:])
                post.append((idx + 4, rb_fn))

            def fill_one():
                nonlocal pe_ns
                if fi[0] < len(fq):
                    cost, fn, st = fq[fi[0]]
                    if st:
                        service_post()
                    fi[0] += 1
                    fn()
                    pe_ns += cost

            for idx, u in enumerate(units):
                cur[0] = idx
                emit_S(u)
                if idx > 0:
                    fill_one()
                    fill_one()
                    prev = units[idx - 1]
                    emit_A(prev)
                    if prev[1] == nt - 1:
                        krt_end(prev[0], idx)
                fill()
            cur[0] = len(units)
            emit_A(units[-1])
            krt_end(3, len(units))
            # close any open filler chain, then flush deferred rbs + rest
            while fi[0] < len(fq) and not fq[fi[0]][2]:
                cost, fn, st = fq[fi[0]]
                fi[0] += 1
                fn()
            service_post(force=True)
            while fi[0] < len(fq):
                cost, fn, st = fq[fi[0]]
                fi[0] += 1
                fn()

        # ---- shared LN helpers ----
        def ln_stat_begin(sp):
            if sp is psS:
                ps_sumA = psS.tile([1, 512], F32, tag="ps_s", name="ps_sumA")
                ps_sumB = psS.tile([1, 512], F32, tag="ps_s", name="ps_sumB")
            else:
                ps_sumA = psA.tile([1, 512], F32, tag="psa", name="ps_sumA")
                ps_sumB = psA.tile([1, 512], F32, tag="psa", name="ps_sumB")
            return ps_sumA, ps_sumB

        def ln_stat_tile(ps_sumA, ps_sumB, src, i):
            tb = cast_p.tile([128, 512], BF16, name="tb")
            nc.vector.tensor_copy(tb[:], src[:])
            nc.tensor.matmul(ps_sumA[:], ones_col_b[:], tb[:],
                             start=(i == 0), stop=(i == DT - 1))
            sqt = sq_p.tile([128, 512], BF16, name="sqt")
            nc.scalar.activation(sqt[:], src[:], AF.Square)
            nc.tensor.matmul(ps_sumB[:], ones_col_b[:], sqt[:],
                             start=(i == 0), stop=(i == DT - 1))

        def ln_stat_finish(ps_sumA, ps_sumB):
            st = strow_p.tile([1, 3 * 512], F32, tag="st", name="st")
            sA, sB2, sC = st[:, 0:512], st[:, 512:1024], st[:, 1024:1536]
            nc.scalar.activation(sA, ps_sumA[:], AF.Copy, scale=1.0 / D)  # u
            nc.scalar.activation(sB2, ps_sumB[:], AF.Identity,
                                 bias=eps_sb[:], scale=1.0 / D)   # msq+eps
            nc.scalar.activation(sC, sA, AF.Square)               # u^2
            nc.vector.tensor_sub(sB2, sB2, sC)                    # var
            nc.vector.reciprocal_approx_fast(sC, sB2)             # 1/var
            nc.scalar.activation(sB2, sC, AF.Sqrt)                # rstd
            nc.vector.tensor_mul(sC, sA, sB2)                     # u*rstd
            rstd_ps = psA.tile([128, 512], F32, tag="psa", name="rstd_ps")
            nc.tensor.matmul(rstd_ps[:], ones_row_f[:], sB2,
                             start=True, stop=True)
            urstd_ps = psA.tile([128, 512], F32, tag="psa", name="urstd_ps")
            nc.tensor.matmul(urstd_ps[:], ones_row_f[:], sC,
                             start=True, stop=True)
            urstd_sb = ubc_p.tile([128, 512], F32, name="urstd_sb")
            nc.vector.tensor_copy(urstd_sb[:], urstd_ps[:])
            return rstd_ps, urstd_sb

        def ln_stats(src_t, sp):
            pA, pB = ln_stat_begin(sp)
            for i in range(DT):
                ln_stat_tile(pA, pB, src_t[i], i)
            return ln_stat_finish(pA, pB)

        def layernorm_to_bf16(src_t, g_sb, b_sb, use_gb, sp):
            rstd_ps, urstd_ps = ln_stats(src_t, sp)
            out_t = []
            for i in range(DT):
                tmpn = tmpn_p.tile([128, 512], F32, name="tmpn")
                nc.vector.tensor_mul(tmpn[:], src_t[i][:], rstd_ps[:])
                nb = nTb_p.tile([128, 512], BF16, tag="nTb", name="nb")
                nc.vector.tensor_sub(nb[:], tmpn[:], urstd_ps[:])
                if use_gb:
                    nc.vector.tensor_scalar(nb[:], nb[:], g_sb[:, i:i + 1],
                                            b_sb[:, i:i + 1], OP.mult, OP.add)
                out_t.append(nb)
            return out_t

        # ============== B: full-width MLP for owned chunk k ================
        def B_res(k):
            """Residual loads + adds (no PE).  Returns t1 tiles."""
            ro = rs_out[k][:].rearrange("(i p) q -> i p q", p=128)
            t1_t = []
            for i in range(DT):
                rob = rob_p.tile([128, 512], BF16, name="rob")
                nc.gpsimd.dma_start(out=rob[:], in_=ro[i])
                xf2 = xf2_p.tile([128, 512], F32, name="xf2")
                nc.gpsimd.dma_start(out=xf2[:], in_=xo[DT * k + i])
                t1 = t1_p.tile([128, 512], F32, name="t1")
                nc.gpsimd.tensor_add(t1[:], rob[:], xf2[:])
                if use_projb:
                    nc.gpsimd.tensor_scalar_add(t1[:], t1[:],
                                                projb_sb[:, i:i + 1])
                t1_t.append(t1)
            return t1_t

        def load_wf(fg):
            pair = []
            for half in range(2):
                t_ = wf_p.tile([128, 4 * 512], BF16, name="wfh")
                nc.sync.dma_start(out=t_[:], in_=wfc[fg, half])
                pair.append(t_)
            return pair

        def B(k, nTb_t, wf_pre=None):
            yield

            # ---- fc + gelu (weights double-buffered one group ahead) ----
            gT_t = []
            pre = list(wf_pre) if wf_pre else [load_wf(0), load_wf(1)]
            for fg in range(8):
                wfh = pre.pop(0)
                if fg + 2 < 8:
                    pre.append(load_wf(fg + 2))
                for ct in range(4):
                    f = 4 * fg + ct
                    ps = psM.tile([128, 512], F32, tag="mm", name="ps_fc")
                    for d in range(DT):
                        w = wfh[d // 4]
                        dd = d % 4
                        nc.tensor.matmul(
                            ps[:],
                            w[:, 512 * dd + 128 * ct:512 * dd + 128 * (ct + 1)],
                            nTb_t[d][:], start=(d == 0), stop=(d == DT - 1))
                    gt = gT_p.tile([128, 512], BF16, name="gt")
                    nc.scalar.activation(gt[:], ps[:], AF.Gelu_apprx_tanh,
                                         bias=fcb_sb[:, f:f + 1])
                    gT_t.append(gt)
                yield

            # ---- cproj with LN2 stats incrementally ----
            dumm = strow_p.tile([1, 3 * 512], F32, tag="st", name="st")
            nc.scalar.activation(dumm[:, 0:1], gT_t[-1][0:1, 0:1],
                                 AF.Sqrt)
            mar_t = []
            pA2, pB2 = ln_stat_begin(psS)
            for p4 in range(4):
                wcq = []
                for qtr in range(4):
                    t_ = wc_p.tile([128, 8 * 256], BF16, name="wcq")
                    nc.sync.dma_start(out=t_[:], in_=wcp[p4, qtr])
                    wcq.append(t_)
                for ci in range(2):
                    dct = 2 * p4 + ci
                    ps = psM.tile([128, 512], F32, tag="mm", name="ps_cp")
                    for f in range(FT):
                        w = wcq[f // 8]
                        fi_ = f % 8
                        nc.tensor.matmul(
                            ps[:],
                            w[:, 256 * fi_ + 128 * ci:256 * fi_ + 128 * (ci + 1)],
                            gT_t[f][:], start=(f == 0), stop=(f == FT - 1))
                    m2 = t1_p.tile([128, 512], F32, name="t1")
                    nc.vector.tensor_add(m2[:], ps[:], nTb_t[dct][:])
                    if use_cprojb:
                        nc.vector.tensor_scalar_add(
                            m2[:], m2[:], cprojb_sb[:, dct:dct + 1])
                    mar_t.append(m2)
                    ln_stat_tile(pA2, pB2, m2, dct)
                yield

            # ---- LN2 -> output ----
            rstd_ps, urstd_ps = ln_stat_finish(pA2, pB2)
            for i in range(DT):
                eng = nc.vector
                ht = hT_p.tile([128, 512], F32, tag="hT", name="ht")
                eng.tensor_mul(ht[:], mar_t[i][:], rstd_ps[:])
                eng.tensor_sub(ht[:], ht[:], urstd_ps[:])
                if use_g2b2:
                    eng.tensor_scalar(ht[:], ht[:], g2_sb[:, i:i + 1],
                                      b2_sb[:, i:i + 1],
                                      OP.mult, OP.add)
                nc.sync.dma_start(out=out[DT * k + i], in_=ht[:])
            yield

        def run(g):
            for _ in g:
                pass

        def trigger_rs(j):
            nc.gpsimd.collective_compute(
                "ReduceScatter", OP.add, replica_groups=PAIRS,
                ins=[rs_in[j][:].opt()], outs=[rs_out[j][:].opt()])

        # =========================== schedule ==============================
        # prologue: QKV(0) as a solid block
        for cost, fn, st in qkv_steps(0):
            fn()

        att_chunk(0, qkv_steps(1))
        att_chunk(1, qkv_steps(2) + proj_steps(0))
        # proj(1) right after att(1) so RS0 can fire early
        for cost, fn, st in proj_steps(1):
            fn()
        trigger_rs(0)
        att_chunk(2, qkv_steps(3))
        t1_b0_box = [None]

        def b0_res_steps():
            def f():
                t1_b0_box[0] = B_res(0)
            return [(0, f, True)]
        att_chunk(3, proj_steps(2) + b0_res_steps())
        # LN1(B0) stats before proj(3): scalar finish chain, stat->bcast
        # latency and the normalize DVE passes all hide under proj(3).
        # tile_wait_until keeps the scheduler from hoisting these RS-gated
        # ops into earlier queue slots where they would block the engine.
        with tc.tile_wait_until(0.255):
            pA1, pB1 = ln_stat_begin(psS)
            for i in range(DT):
                ln_stat_tile(pA1, pB1, t1_b0_box[0][i], i)
        p3 = proj_steps(3, drain_scalar=True)
        for cost, fn, st in p3[:12]:
            fn()
        with tc.tile_wait_until(0.258):
            rstd1, urstd1 = ln_stat_finish(pA1, pB1)
        nTb0 = []

        def norm0(i):
            with tc.tile_wait_until(0.26):
                eng = nc.vector
                tmpn = tmpn_p.tile([128, 512], F32, name="tmpn")
                eng.tensor_mul(tmpn[:], t1_b0_box[0][i][:], rstd1[:])
                nb = nTb_p.tile([128, 512], BF16, tag="nTb", name="nb")
                eng.tensor_sub(nb[:], tmpn[:], urstd1[:])
                if use_g1b1:
                    eng.tensor_scalar(nb[:], nb[:], g1_sb[:, i:i + 1],
                                      b1_sb[:, i:i + 1],
                                      OP.mult, OP.add)
                nTb0.append(nb)
        ni = 0
        for si, (cost, fn, st) in enumerate(p3[12:]):
            fn()
            if si % 2 == 1 and ni < DT:
                norm0(ni)
                ni += 1
        while ni < DT:
            norm0(ni)
            ni += 1
        # release attention weights, open MLP weight pools, then prefetch
        # fc(B0) weights ahead of the collective's DMA traffic
        actx.close()
        wf_p = pool("wf_p", 5)          # fc weight half-groups [128, 2048]
        gT_p = pool("gT_p", 32)         # gelu outputs bf16
        wc_p = pool("wc_p", 3)          # cproj weight quarter [128, 2048]
        wf_pre0 = [load_wf(0), load_wf(1)]
        trigger_rs(1)

        b0 = B(0, nTb0, wf_pre0)
        for _ in range(10):
            next(b0)      # (noop), fc fg0..7, cproj p4-0
        with tc.tile_wait_until(0.41):
            nTb1 = layernorm_to_bf16(B_res(1), g1_sb, b1_sb, use_g1b1, psA)
        b1 = B(1, nTb1)
        next(b1)          # noop yield
        for _ in range(3):
            next(b0)      # cproj p4 1-3
        for _ in range(2):
            next(b1)      # fc fg0, fg1 — cover b0's LN2 finish
        run(b0)           # LN2(B0) finish + normalize + out
        run(b1)           # fc rest, cproj, LN2

    nc.compile()
    return nc


_cache = {}


def _get_program(flags):
    if flags not in _cache:
        _cache[flags] = _build(*flags)
    return _cache[flags]


def _pack(a, rows, cols):
    """[R, C] -> [R//rows * C//cols, rows, cols], row-tile-major."""
    R, C = a.shape
    return np.ascontiguousarray(
        a.reshape(R // rows, rows, C // cols, cols).transpose(0, 2, 1, 3)
        .reshape(-1, rows, cols))


def _swap(p, n_rt, n_ct):
    """_pack gives (row-tile, col-tile) order; swap to (col, row)."""
    t = p.reshape(n_rt, n_ct, p.shape[1], p.shape[2])
    return np.ascontiguousarray(
        t.transpose(1, 0, 2, 3).reshape(-1, p.shape[1], p.shape[2]))


def _rowpack(tiles):
    """[N, 128, C] tile list -> [128, N*C] contiguous-row layout."""
    n, p, c = tiles.shape
    return np.ascontiguousarray(tiles.transpose(1, 0, 2).reshape(p, n * c))


def _prepare_inputs(inputs):
    x = np.asarray(inputs["x"], np.float32)
    weight = float(np.asarray(inputs["weight"]).reshape(-1)[0])
    linear_w = np.asarray(inputs["linear_w"], np.float32)
    linear_b = np.asarray(inputs["linear_b"], np.float32)
    proj_w = np.asarray(inputs["proj_w"], np.float32)
    proj_b = np.asarray(inputs["proj_b"], np.float32)
    ln1_g = np.asarray(inputs["ln1_g"], np.float32)
    ln1_b = np.asarray(inputs["ln1_b"], np.float32)
    fc_w = np.asarray(inputs["fc_w"], np.float32)
    fc_b = np.asarray(inputs["fc_b"], np.float32)
    cproj_w = np.asarray(inputs["cproj_w"], np.float32)
    cproj_b = np.asarray(inputs["cproj_b"], np.float32)
    ln2_g = np.asarray(inputs["ln2_g"], np.float32)
    ln2_b = np.asarray(inputs["ln2_b"], np.float32)
    idx = np.asarray(inputs["idx"]).astype(np.int64).reshape(-1)
    forcing = bool(np.asarray(inputs["attn_forcing"]).reshape(-1)[0])

    flags = (
        bool(linear_b[:2048].any()),      # use_bqk
        bool(linear_b[2048:].any()),      # use_bv
        bool(proj_b.any()),
        bool(cproj_b.any()),
        bool((ln1_g != 1).any() or ln1_b.any()),
        bool((ln2_g != 1).any() or ln2_b.any()),
    )

    if forcing:
        lnw = float(np.log(weight)) if weight >= 1e-37 else -1e9
        pos = np.arange(S)
        lna_all = np.where(pos[None, :] >= idx[:, None], lnw, 0.0) \
            .astype(np.float32)
    else:
        lna_all = np.zeros((B, S), np.float32)

    # strict upper triangle gets -1e9 (causal mask via matmul accumulate):
    # out[k, qq] += trin[qq, k] must be -1e9 when k > qq.
    trin_np = np.where(np.arange(128)[None, :] > np.arange(128)[:, None],
                       np.float32(-1e9), np.float32(0.0)).astype(BF)
    eye_np = np.eye(128, dtype=np.float32).astype(BF)
    sel2_np = np.zeros((1, 256), np.float32)
    sel2_np[0, 0:64] = 1.0       # even-head recip -> partitions 0..63
    sel2_np[0, 128 + 64:] = 1.0  # odd-head recip -> partitions 64..127
    sel2_np = sel2_np.astype(BF)

    # ---- global (all-core) MLP weights ----
    wfc_p = _swap(_pack(fc_w.astype(BF), 128, 512), DT, 8)   # (fg, d)
    wfc_g = np.ascontiguousarray(
        wfc_p.reshape(8, 2, 4, 128, 512).transpose(0, 1, 3, 2, 4)
        .reshape(8, 2, 128, 4 * 512))
    wcp_p = _swap(_pack(cproj_w.astype(BF), 128, 256), FT, 4)  # (p4, f)
    wcp_g = np.ascontiguousarray(
        wcp_p.reshape(4, 4, 8, 128, 256).transpose(0, 1, 3, 2, 4)
        .reshape(4, 4, 128, 8 * 256))

    in_maps = []
    for core in range(N_CORES):
        b, r = core // 2, core % 2
        q_cols = slice(512 * r, 512 * (r + 1))
        k_cols = slice(1024 + 512 * r, 1024 + 512 * (r + 1))
        v_cols = slice(2048 + 512 * r, 2048 + 512 * (r + 1))
        xT = np.ascontiguousarray(x[b].T)                       # [D, S]
        wqk_full = np.concatenate([linear_w[:, q_cols], linear_w[:, k_cols]],
                                  axis=1)                       # [D, 1024]

        xq_t = _pack(xT, 128, 512)                  # (d, c): index d*NCH+c
        xq_dc = xq_t.reshape(DT, NCH, 128, 512)
        xqb = np.ascontiguousarray(
            xq_dc.transpose(1, 0, 2, 3).reshape(NCH, 2, 4, 128, 512)
            .transpose(0, 1, 3, 2, 4).reshape(NCH, 2, 128, 4 * 512)
        ).astype(BF)
        own = [r, 2 + r]
        xo_np = np.ascontiguousarray(
            xq_dc[:, own].transpose(1, 0, 2, 3).reshape(2 * DT, 128, 512))

        in_maps.append({
            "xqb": xqb,
            "xo": xo_np,
            "wqk": _rowpack(_swap(_pack(wqk_full.astype(BF), 128, 512),
                                  8, 2)),
            "bqk": np.ascontiguousarray(
                np.concatenate([linear_b[q_cols], linear_b[k_cols]])),
            "wv": _rowpack(_pack(linear_w[:, v_cols].astype(BF), 128, 512)),
            "bv": np.ascontiguousarray(linear_b[v_cols]).astype(BF),
            "wproj": _rowpack(_swap(_pack(proj_w[512 * r:512 * (r + 1), :]
                                          .astype(BF), 128, 512), 4, 2)),
            "projb": proj_b,
            "wfc": wfc_g, "fcb": fc_b,
            "wcp": wcp_g, "cprojb": cproj_b,
            "g1": ln1_g, "b1": ln1_b, "g2": ln2_g, "b2": ln2_b,
            "lna": lna_all[b],
            "trin": trin_np,
            "eye": eye_np,
            "sel2": sel2_np,
        })
    return flags, in_maps


def _unpack_out(o_pair):
    """o_pair: [out_core0, out_core1] each [2*DT, 128, 512] -> [S, D]."""
    hT = np.empty((D, S), np.float32)
    for r in range(2):
        o = o_pair[r].reshape(2, DT, 128, 512)
        for k, c in enumerate([r, 2 + r]):
            hT[:, CH * c:CH * (c + 1)] = o[k].reshape(D, 512)
    return hT.T


def _run(inputs, trace=False):
    flags, in_maps = _prepare_inputs(inputs)
    nc = _get_program(flags)
    r = run_bass_kernel_spmd(nc, in_maps, core_ids=list(range(N_CORES)),
                             trace=trace)
    outs = np.stack(
        [_unpack_out([r.results[2 * b]["out"], r.results[2 * b + 1]["out"]])
         for b in range(B)], axis=0).astype(np.float32)
    return outs, r


def kernel(**inputs) -> np.ndarray:
    outs, _ = _run(inputs, trace=False)
    return outs


# revision 59
# speedup vs baseline: 1.0298x; 1.0043x over previous
"""Dense transformer block on 8 TRN2 NeuronCores.

Sharding: data-parallel over batch (4 pairs of cores). Within each pair:
  - Attention is Megatron head-parallel (8 heads per core, all tokens).
  - Post-attention (residual+LN1+MLP+LN2) is chunk-parallel: partial sums
    of the attention projection for two 512-token chunks are combined with
    one pairwise ReduceScatter per chunk-pair; core r owns chunks {r, 2+r}
    and runs the MLP full-width locally (no second collective).

Device schedule is built around keeping the PE (tensor engine)
continuously busy so it stays at its max p-state clock:
  - Scores for both heads of a (krt, t) unit land in ONE 2-bank PSUM tile
    [128, 1024] and are exponentiated by a single wide ACT instruction.
  - The causal mask is applied by a tiny extra matmul (tri_neg @ I128)
    accumulated into the score PSUM *before* the exp, so there is no
    vector-engine mask pass between exp and attn@V.
  - The PE stream is software-pipelined one unit ahead (S(u+1) issues
    before A(u)), and QKV matmuls of chunk c+1 plus the projection of
    chunk c are woven into attention of chunk c as filler so the PE never
    waits on the Scalar engine's exp stream.
  - Softmax denominators ride attn@V as an extra ones-column of V; the
    two den rows are gathered to partitions 0..1 with a tiny DMA,
    reciprocated with the fast DVE approx, and broadcast across
    partitions with one sel2 @ rec matmul whose emission is deferred a
    few units so the PE stream never waits on the den chain.
  - RS-gated work (residual loads/adds after each ReduceScatter) runs on
    the otherwise idle gpsimd queue, and tile_wait_until hints keep the
    scheduler from hoisting its consumers into engine-queue slots where
    they would head-of-line block attention.
"""

import numpy as np
import ml_dtypes

import concourse.bacc as bacc
import concourse.mybir as mybir
import concourse.tile as tile
from concourse.bass_utils import run_bass_kernel_spmd

F32 = mybir.dt.float32
BF16 = mybir.dt.bfloat16
AF = mybir.ActivationFunctionType
OP = mybir.AluOpType

B, S, D, H, HD, FF = 4, 2048, 1024, 16, 64, 4096
N_CORES = 8
PAIRS = [[0, 1], [2, 3], [4, 5], [6, 7]]
CH = 512                 # tokens per chunk
NCH = S // CH            # 4
DT = D // 128            # 8 d-tiles
FT = FF // 128           # 32 f-tiles
KT = S // 128            # 16 kpos tiles
EPS = 1e-5
BF = ml_dtypes.bfloat16

PE_NS = 0.42             # ns per output column at max clock
EXP_NS = 0.833           # scalar ns per column


def _build(use_bqk, use_bv, use_projb, use_cprojb, use_g1b1, use_g2b2):
    nc = bacc.Bacc("TRN2", target_bir_lowering=False, debug=False,
                   enable_asserts=True, num_devices=N_CORES)

    # ---- DRAM inputs (tile-packed on host) ----
    xqb = nc.dram_tensor("xqb", [NCH, 2, 128, 4 * 512], BF16,
                         kind="ExternalInput")          # bf16 x^T (c, half)
    xo = nc.dram_tensor("xo", [2 * DT, 128, 512], F32,
                        kind="ExternalInput")           # f32 x^T own chunks
    wqk = nc.dram_tensor("wqk", [128, 16 * 512], BF16, kind="ExternalInput")
    bqk = nc.dram_tensor("bqk", [1024], F32, kind="ExternalInput")
    wv = nc.dram_tensor("wv", [128, 8 * 512], BF16, kind="ExternalInput")
    bv = nc.dram_tensor("bv", [512], BF16, kind="ExternalInput")
    wproj = nc.dram_tensor("wproj", [128, 8 * 512], BF16,
                           kind="ExternalInput")
    projb = nc.dram_tensor("projb", [D], F32, kind="ExternalInput")
    wfc = nc.dram_tensor("wfc", [8, 2, 128, 4 * 512], BF16,
                         kind="ExternalInput")          # (fg, half) x (d,q)
    fcb = nc.dram_tensor("fcb", [FF], F32, kind="ExternalInput")
    wcp = nc.dram_tensor("wcp", [4, 4, 128, 8 * 256], BF16,
                         kind="ExternalInput")          # (p4, qtr) x (f,q)
    cprojb = nc.dram_tensor("cprojb", [D], F32, kind="ExternalInput")
    g1 = nc.dram_tensor("g1", [D], F32, kind="ExternalInput")
    b1 = nc.dram_tensor("b1", [D], F32, kind="ExternalInput")
    g2 = nc.dram_tensor("g2", [D], F32, kind="ExternalInput")
    b2 = nc.dram_tensor("b2", [D], F32, kind="ExternalInput")
    lna = nc.dram_tensor("lna", [S], F32, kind="ExternalInput")
    trin = nc.dram_tensor("trin", [128, 128], BF16, kind="ExternalInput")
    eye = nc.dram_tensor("eye", [128, 128], BF16, kind="ExternalInput")
    sel2 = nc.dram_tensor("sel2", [1, 256], BF16, kind="ExternalInput")
    # output: own chunks (k, i) tiles; host reassembles
    out = nc.dram_tensor("out", [2 * DT, 128, 512], F32,
                         kind="ExternalOutput")

    from contextlib import ExitStack
    with tile.TileContext(nc) as tc, ExitStack() as ctx:
        def pool(name, bufs, space="SBUF"):
            return ctx.enter_context(
                tc.tile_pool(name=name, bufs=bufs, space=space))

        const = pool("const", 1)
        xb_p = pool("xb_p", 2)          # bf16 x half-chunks [128, 2048]
        qTb_p = pool("qTb_p", 6)
        pt_p = pool("pt_p", 4)          # exp outputs [128, 1024] bf16
        attnTb_p = pool("attnTb_p", 8)
        dd_p = pool("dd_p", 2)          # den recip rows f32
        ddb_p = pool("ddb_p", 2)        # den recip rows bf16
        t64_p = pool("t64_p", 2)        # odd-head attn staging for DMA move
        ai_p = pool("ai_p", 2)          # proj partial bf16 tiles
        rob_p = pool("rob_p", 2)        # bf16 rs_out staging
        t1_p = pool("t1_p", 10)         # B: residual tiles f32 (t1 AND n+m)
        xf2_p = pool("xf2_p", 2)
        cast_p = pool("cast_p", 1)      # LN bf16 casts
        sq_p = pool("sq_p", 1)
        strow_p = pool("strow_p", 1)
        nTb_p = pool("nTb_p", 8)        # bf16 n tiles (fc rhs + s3 residual)
        tmpn_p = pool("tmpn_p", 1)
        hT_p = pool("hT_p", 3)
        ubc_p = pool("ubc_p", 1)        # u*rstd SBUF copy
        psS = pool("psS", 2, "PSUM")    # [128,1024] score pair tiles
        psA = pool("psA", 2, "PSUM")    # [128,512] av accumulators + bcast
        psM = pool("psM", 2, "PSUM")    # [128,512] general matmul tiles
        dram = pool("dram", 2, "DRAM")

        # ---- constants ----
        kt_sb = const.tile([128, 4 * S], BF16, name="kt_sb")
        kt_v = kt_sb[:].rearrange("p (r q) -> p r q", q=S)
        v_sb = const.tile([128, KT * 520], BF16, name="v_sb")
        v_v = v_sb[:].rearrange("p (t e) -> p t e", e=520)
        # ones column of V (den rides attn@V as the 65th row), set once
        v_h = v_sb[:].rearrange("p (t h e) -> p t h e", h=8, e=65)
        nc.vector.memset(v_h[:, :, :, 64:65], 1.0)

        # prefetch chunk 0's x before the big weight DMAs
        xh0 = []
        for half in range(2):
            t_ = xb_p.tile([128, 4 * 512], BF16, name="xh0")
            nc.sync.dma_start(out=t_[:], in_=xqb[0, half])
            xh0.append(t_)

        # ---- resident attention weights (pool closed before B phase so
        # the MLP weight pools can reuse its SBUF space) ----
        actx = ExitStack()
        wres = actx.enter_context(
            tc.tile_pool(name="wres", bufs=1, space="SBUF"))
        wqk_sb = wres.tile([128, 16 * 512], BF16, name="wqk_sb")
        nc.sync.dma_start(out=wqk_sb[:, 0:8 * 512], in_=wqk[:, 0:8 * 512])
        nc.scalar.dma_start(out=wqk_sb[:, 8 * 512:], in_=wqk[:, 8 * 512:])
        wqk_t = [wqk_sb[:, 512 * i:512 * (i + 1)] for i in range(16)]
        wv_sb = wres.tile([128, 8 * 512], BF16, name="wv_sb")
        nc.scalar.dma_start(out=wv_sb[:], in_=wv[:])
        wv_t = [wv_sb[:, 512 * i:512 * (i + 1)] for i in range(8)]
        wpr_sb = wres.tile([128, 8 * 512], BF16, name="wpr_sb")
        nc.scalar.dma_start(out=wpr_sb[:], in_=wproj[:])
        wpr_t = [wpr_sb[:, 512 * i:512 * (i + 1)] for i in range(8)]

        trin_sb = const.tile([128, 128], BF16, name="trin_sb")
        nc.scalar.dma_start(out=trin_sb[:], in_=trin[:])
        eye_sb = const.tile([128, 128], BF16, name="eye_sb")
        nc.scalar.dma_start(out=eye_sb[:], in_=eye[:])
        sel2b_sb = const.tile([2, 128], BF16, name="sel2b_sb")
        nc.scalar.dma_start(out=sel2b_sb[:],
                            in_=sel2.rearrange("o (a b) -> (o a) b", a=2))
        lna_sb = const.tile([128, KT], F32, name="lna_sb")
        nc.scalar.dma_start(out=lna_sb[:],
                            in_=lna.rearrange("(t p) -> p t", p=128))
        ones_col_b = const.tile([128, 1], BF16, name="ones_col_b")
        nc.vector.memset(ones_col_b[:], 1.0)
        ones_row_f = const.tile([1, 128], F32, name="ones_row_f")
        nc.vector.memset(ones_row_f[:], 1.0)
        eps_sb = const.tile([1, 1], F32, name="eps_sb")
        nc.vector.memset(eps_sb[:], EPS)
        fcb_sb = const.tile([128, FT], F32, name="fcb_sb")
        nc.scalar.dma_start(out=fcb_sb[:],
                            in_=fcb.rearrange("(i p) -> p i", p=128))

        def vec8(name, t):
            sb = const.tile([128, DT], F32, name=name)
            nc.scalar.dma_start(out=sb[:],
                                in_=t.rearrange("(i p) -> p i", p=128))
            return sb

        bqk_sb = vec8("bqk_sb", bqk) if use_bqk else None
        projb_sb = vec8("projb_sb", projb) if use_projb else None
        cprojb_sb = vec8("cprojb_sb", cprojb) if use_cprojb else None
        g1_sb = vec8("g1_sb", g1) if use_g1b1 else None
        b1_sb = vec8("b1_sb", b1) if use_g1b1 else None
        g2_sb = vec8("g2_sb", g2) if use_g2b2 else None
        b2_sb = vec8("b2_sb", b2) if use_g2b2 else None
        if use_bv:
            ones_row_b = const.tile([1, 128], BF16, name="ones_row_b")
            nc.vector.memset(ones_row_b[:], 1.0)
            bv_sb = const.tile([1, 512], BF16, name="bv_sb")
            nc.sync.dma_start(out=bv_sb[:],
                              in_=bv.rearrange("(o q) -> o q", o=1))

        # ---- ReduceScatter buffers (bf16 payload halves the wire) ----
        rs_in = [dram.tile([2 * D, 512], BF16, tag=f"rsi{j}",
                           name=f"rs_in{j}") for j in range(2)]
        rs_out = [dram.tile([D, 512], BF16, tag=f"rso{j}",
                            name=f"rs_out{j}") for j in range(2)]

        qtb_tiles = {0: [None] * 4, 1: [None] * 4, 2: [None] * 4,
                     3: [None] * 4}
        attn_tiles = {c: [None] * 4 for c in range(4)}

        # ================= QKV filler steps for chunk c ====================
        # Steps are (pe_cost_ns, closure, chain_start).  chain_start marks
        # safe points where a deferred rb matmul may allocate from the
        # shared psM ring without colliding with an open accumulation chain.
        def qkv_steps(c):
            tok = slice(CH * c, CH * (c + 1))
            steps = []
            if c == 0:
                xh = xh0
            else:
                xh = [None, None]

                def load(half):
                    def f():
                        t_ = xb_p.tile([128, 4 * 512], BF16, name="xh")
                        nc.sync.dma_start(out=t_[:], in_=xqb[c, half])
                        xh[half] = t_
                    return f
                steps.append((0, load(0), True))
                steps.append((0, load(1), True))

            def xtb(d):
                return xh[d // 4][:, 512 * (d % 4):512 * (d % 4 + 1)]

            # 8 QK chains (i<4: Q -> qTb, else K -> kt_v)
            for cc in range(2):
                for ct in range(4):
                    i = 4 * cc + ct
                    box = [None]

                    def mk(d, i=i, cc=cc, ct=ct, box=box):
                        def f():
                            if d == 0:
                                box[0] = psM.tile([128, 512], F32, tag="mm",
                                                  name="ps_qk")
                            nc.tensor.matmul(
                                box[0][:],
                                wqk_t[8 * cc + d][:, 128 * ct:128 * (ct + 1)],
                                xtb(d), start=(d == 0), stop=(d == DT - 1))
                            if d == DT - 1:
                                if i < 4:
                                    dt_ = qTb_p.tile([128, 512], BF16,
                                                     name="qTb")
                                    qtb_tiles[c][i] = dt_
                                    dest = dt_[:]
                                else:
                                    dest = kt_v[:, i - 4, tok]
                                if use_bqk:
                                    nc.vector.tensor_scalar_add(
                                        dest, box[0][:], bqk_sb[:, i:i + 1])
                                else:
                                    nc.vector.tensor_copy(dest, box[0][:])
                        return f
                    for d in range(DT):
                        steps.append((512 * PE_NS, mk(d), d == 0))

            # 4 V chains
            for tt in range(4):
                tg = 4 * c + tt
                box = [None]

                def mkv(d, tt=tt, tg=tg, box=box):
                    def f():
                        if d == 0:
                            box[0] = psM.tile([128, 512], F32, tag="mm",
                                              name="ps_v")
                        nc.tensor.matmul(
                            box[0][:], xtb(d)[:, 128 * tt:128 * (tt + 1)],
                            wv_t[d], start=(d == 0),
                            stop=(d == DT - 1 and not use_bv))
                        if d == DT - 1:
                            if use_bv:
                                nc.tensor.matmul(box[0][:], ones_row_b[:],
                                                 bv_sb[:], start=False,
                                                 stop=True)
                            ps_h = box[0][:].rearrange(
                                "p (h e) -> p h e", e=64)
                            nc.vector.tensor_copy(
                                v_h[:, tg, :, 0:64], ps_h[:])
                    return f
                for d in range(DT):
                    steps.append((512 * PE_NS, mkv(d), d == 0))
            return steps

        # ================= proj steps for chunk c ==========================
        def proj_steps(c, drain_scalar=False):
            blk = c % 2
            ri = rs_in[c // 2][:] \
                .rearrange("(k i p) q -> k i p q", k=2, p=128)
            steps = []
            for cc in range(2):
                for ct in range(4):
                    dct = 4 * cc + ct
                    box = [None]

                    def mk(r, cc=cc, ct=ct, dct=dct, box=box):
                        def f():
                            if r == 0:
                                box[0] = psM.tile([128, 512], F32, tag="mm",
                                                  name="ps_pr")
                            nc.tensor.matmul(
                                box[0][:],
                                wpr_t[4 * cc + r][:, 128 * ct:128 * (ct + 1)],
                                attn_tiles[c][r][:], start=(r == 0),
                                stop=(r == 3))
                            if r == 3:
                                ai = ai_p.tile([128, 512], BF16, name="ai")
                                if drain_scalar:
                                    nc.scalar.activation(ai[:], box[0][:],
                                                         AF.Copy)
                                else:
                                    nc.vector.tensor_copy(ai[:], box[0][:])
                                nc.sync.dma_start(out=ri[blk, dct], in_=ai[:])
                        return f
                    for r in range(4):
                        steps.append((512 * PE_NS, mk(r), r == 0))
            return steps

        # ================= attention for chunk c ===========================
        def att_chunk(c, filler):
            """Pipelined attention units with filler weave.  filler is a
            list of (pe_cost, fn, chain_start); consumed front-to-back."""
            nt = 4 * (c + 1)
            units = [(krt, t) for krt in range(4) for t in range(nt)]
            qtb = qtb_tiles[c]
            pe_ns = 0.0
            sc_ns = 0.0
            fq = list(filler)
            fi = [0]
            cur = [0]
            state = {}      # krt -> (pa0, pa1)
            P_of = {}       # unit -> (P, pt, qo)
            post = []       # deferred (due_idx, fn) — rb matmuls

            def service_post(force=False):
                nonlocal pe_ns
                while post and (force or post[0][0] <= cur[0]):
                    post.pop(0)[1]()
                    pe_ns += 1024 * PE_NS

            def fill():
                nonlocal pe_ns
                while fi[0] < len(fq) and pe_ns < sc_ns:
                    cost, fn, st = fq[fi[0]]
                    if st:
                        service_post()
                    fi[0] += 1
                    fn()
                    pe_ns += cost
                if fi[0] >= len(fq):
                    service_post()

            def emit_S(u):
                nonlocal pe_ns, sc_ns
                krt, t = u
                j = t - 4 * c
                qo = 128 * j if j >= 0 else 0
                P = psS.tile([128, 1024], F32, tag="ps_s", name="P")
                for h in range(2):
                    o = 512 * h
                    nc.tensor.matmul(
                        P[:, o + qo:o + 512],
                        kt_v[64 * h:64 * (h + 1), krt, 128 * t:128 * (t + 1)],
                        qtb[krt][64 * h:64 * (h + 1), qo:512],
                        start=True, stop=(j < 0))
                    if j >= 0:
                        nc.tensor.matmul(
                            P[:, o + qo:o + qo + 128], trin_sb[:],
                            eye_sb[:], start=False, stop=True)
                pe_ns += 2 * (512 - qo) * PE_NS + (256 * PE_NS if j >= 0
                                                  else 0)
                pt = pt_p.tile([128, 1024], BF16, name="pt")
                pv = P[:].rearrange("p (z q) -> p z q", q=512)
                tv = pt[:].rearrange("p (z q) -> p z q", q=512)
                nc.scalar.activation(tv[:, :, qo:], pv[:, :, qo:], AF.Exp,
                                     bias=lna_sb[:, t:t + 1], scale=0.125)
                sc_ns += 2 * (512 - qo) * EXP_NS + 250
                P_of[u] = (P, pt, qo)

            def emit_A(u):
                nonlocal pe_ns
                krt, t = u
                P, pt, qo = P_of.pop(u)
                if t == 0:
                    pa0 = psA.tile([128, 512], F32, tag="psa", name="pa0")
                    pa1 = psA.tile([128, 512], F32, tag="psa", name="pa1")
                    state[krt] = (pa0, pa1)
                pa0, pa1 = state[krt]
                h0, h1 = 2 * krt, 2 * krt + 1
                nc.tensor.matmul(pa0[0:65, qo:], v_v[:, t, 65 * h0:
                                                      65 * h0 + 65],
                                 pt[:, qo:512], start=(t == 0),
                                 stop=(t == nt - 1))
                nc.tensor.matmul(pa1[0:65, qo:], v_v[:, t, 65 * h1:
                                                      65 * h1 + 65],
                                 pt[:, 512 + qo:1024], start=(t == 0),
                                 stop=(t == nt - 1))
                pe_ns += 2 * (512 - qo) * PE_NS

            def krt_end(krt, idx):
                """attnTb copies + den reciprocal; rb matmul deferred."""
                nonlocal sc_ns
                pa0, pa1 = state.pop(krt)
                at = attnTb_p.tile([128, 512], BF16, name="at")
                attn_tiles[c][krt] = at
                nc.scalar.activation(at[0:64, :], pa0[0:64, :], AF.Copy)
                t64 = t64_p.tile([64, 512], BF16, name="t64")
                nc.scalar.activation(t64[:], pa1[0:64, :], AF.Copy)
                nc.sync.dma_start(out=at[64:128, :], in_=t64[:])
                sc_ns += 2 * 512 * EXP_NS + 500
                dd0 = dd_p.tile([65, 512], F32, tag="dd", name="dd0")
                nc.vector.tensor_copy(dd0[64:65, :], pa0[64:65, :])
                dd1 = dd_p.tile([65, 512], F32, tag="dd", name="dd1")
                nc.vector.tensor_copy(dd1[64:65, :], pa1[64:65, :])
                den2 = ddb_p.tile([2, 512], F32, tag="den2", name="den2")
                nc.sync.dma_start(out=den2[0:1, :], in_=dd0[64:65, :])
                nc.sync.dma_start(out=den2[1:2, :], in_=dd1[64:65, :])
                rec2 = ddb_p.tile([2, 512], F32, tag="rec2", name="rec2")
                nc.vector.reciprocal_approx_fast(rec2[:], den2[:])
                rcb = ddb_p.tile([2, 512], BF16, tag="rcb", name="rcb")
                nc.vector.tensor_copy(rcb[:], rec2[:])

                def rb_fn():
                    rb = psM.tile([128, 512], F32, tag="mm", name="rb")
                    nc.tensor.matmul(rb[:], sel2b_sb[:], rcb[:],
                                     start=True, stop=True)
                    nc.vector.tensor_mul(at[:], at[:], rb[:])
                post.append((idx + 4, rb_fn))

            def fill_one():
                nonlocal pe_ns
                if fi[0] < len(fq):
                    cost, fn, st = fq[fi[0]]
                    if st:
                        service_post()
                    fi[0] += 1
                    fn()
                    pe_ns += cost

            for idx, u in enumerate(units):
                cur[0] = idx
                emit_S(u)
                if idx > 0:
                    fill_one()
                    fill_one()
                    prev = units[idx - 1]
                    emit_A(prev)
                    if prev[1] == nt - 1:
                        krt_end(prev[0], idx)
                fill()
            cur[0] = len(units)
            emit_A(units[-1])
            krt_end(3, len(units))
            # close any open filler chain, then flush deferred rbs + rest
            while fi[0] < len(fq) and not fq[fi[0]][2]:
                cost, fn, st = fq[fi[0]]
                fi[0] += 1
                fn()
            service_post(force=True)
            while fi[0] < len(fq):
                cost, fn, st = fq[fi[0]]
                fi[0] += 1
                fn()

        # ---- shared LN helpers ----
        def ln_stat_begin(sp):
            if sp is psS:
                ps_sumA = psS.tile([1, 512], F32, tag="ps_s", name="ps_sumA")
                ps_sumB = psS.tile([1, 512], F32, tag="ps_s", name="ps_sumB")
            else:
                ps_sumA = psA.tile([1, 512], F32, tag="psa", name="ps_sumA")
                ps_sumB = psA.tile([1, 512], F32, tag="psa", name="ps_sumB")
            return ps_sumA, ps_sumB

        def ln_stat_tile(ps_sumA, ps_sumB, src, i):
            tb = cast_p.tile([128, 512], BF16, name="tb")
            nc.vector.tensor_copy(tb[:], src[:])
            nc.tensor.matmul(ps_sumA[:], ones_col_b[:], tb[:],
                             start=(i == 0), stop=(i == DT - 1))
            sqt = sq_p.tile([128, 512], BF16, name="sqt")
            nc.scalar.activation(sqt[:], src[:], AF.Square)
            nc.tensor.matmul(ps_sumB[:], ones_col_b[:], sqt[:],
                             start=(i == 0), stop=(i == DT - 1))

        def ln_stat_finish(ps_sumA, ps_sumB):
            st = strow_p.tile([1, 3 * 512], F32, tag="st", name="st")
            sA, sB2, sC = st[:, 0:512], st[:, 512:1024], st[:, 1024:1536]
            nc.scalar.activation(sA, ps_sumA[:], AF.Copy, scale=1.0 / D)  # u
            nc.scalar.activation(sB2, ps_sumB[:], AF.Identity,
                                 bias=eps_sb[:], scale=1.0 / D)   # msq+eps
            nc.scalar.activation(sC, sA, AF.Square)               # u^2
            nc.vector.tensor_sub(sB2, sB2, sC)                    # var
            nc.vector.reciprocal_approx_fast(sC, sB2)             # 1/var
            nc.scalar.activation(sB2, sC, AF.Sqrt)                # rstd
            nc.vector.tensor_mul(sC, sA, sB2)                     # u*rstd
            rstd_ps = psA.tile([128, 512], F32, tag="psa", name="rstd_ps")
            nc.tensor.matmul(rstd_ps[:], ones_row_f[:], sB2,
                             start=True, stop=True)
            urstd_ps = psA.tile([128, 512], F32, tag="psa", name="urstd_ps")
            nc.tensor.matmul(urstd_ps[:], ones_row_f[:], sC,
                             start=True, stop=True)
            urstd_sb = ubc_p.tile([128, 512], F32, name="urstd_sb")
            nc.vector.tensor_copy(urstd_sb[:], urstd_ps[:])
            return rstd_ps, urstd_sb

        def ln_stats(src_t, sp):
            pA, pB = ln_stat_begin(sp)
            for i in range(DT):
                ln_stat_tile(pA, pB, src_t[i], i)
            return ln_stat_finish(pA, pB)

        def layernorm_to_bf16(src_t, g_sb, b_sb, use_gb, sp):
            rstd_ps, urstd_ps = ln_stats(src_t, sp)
            out_t = []
            for i in range(DT):
                tmpn = tmpn_p.tile([128, 512], F32, name="tmpn")
                nc.vector.tensor_mul(tmpn[:], src_t[i][:], rstd_ps[:])
                nb = nTb_p.tile([128, 512], BF16, tag="nTb", name="nb")
                nc.vector.tensor_sub(nb[:], tmpn[:], urstd_ps[:])
                if use_gb:
                    nc.vector.tensor_scalar(nb[:], nb[:], g_sb[:, i:i + 1],
                                            b_sb[:, i:i + 1], OP.mult, OP.add)
                out_t.append(nb)
            return out_t

        # ============== B: full-width MLP for owned chunk k ================
        def B_res(k):
            """Residual loads + adds (no PE).  Returns t1 tiles."""
            ro = rs_out[k][:].rearrange("(i p) q -> i p q", p=128)
            t1_t = []
            for i in range(DT):
                rob = rob_p.tile([128, 512], BF16, name="rob")
                nc.gpsimd.dma_start(out=rob[:], in_=ro[i])
                xf2 = xf2_p.tile([128, 512], F32, name="xf2")
                nc.gpsimd.dma_start(out=xf2[:], in_=xo[DT * k + i])
                t1 = t1_p.tile([128, 512], F32, name="t1")
                nc.gpsimd.tensor_add(t1[:], rob[:], xf2[:])
                if use_projb:
                    nc.gpsimd.tensor_scalar_add(t1[:], t1[:],
                                                projb_sb[:, i:i + 1])
                t1_t.append(t1)
            return t1_t

        def load_wf(fg):
            pair = []
            for half in range(2):
                t_ = wf_p.tile([128, 4 * 512], BF16, name="wfh")
                nc.sync.dma_start(out=t_[:], in_=wfc[fg, half])
                pair.append(t_)
            return pair

        def B(k, nTb_t, wf_pre=None):
            yield

            # ---- fc + gelu (weights double-buffered one group ahead) ----
            gT_t = []
            pre = list(wf_pre) if wf_pre else [load_wf(0), load_wf(1)]
            for fg in range(8):
                wfh = pre.pop(0)
                if fg + 2 < 8:
                    pre.append(load_wf(fg + 2))
                for ct in range(4):
                    f = 4 * fg + ct
                    ps = psM.tile([128, 512], F32, tag="mm", name="ps_fc")
                    for d in range(DT):
                        w = wfh[d // 4]
                        dd = d % 4
                        nc.tensor.matmul(
                            ps[:],
                            w[:, 512 * dd + 128 * ct:512 * dd + 128 * (ct + 1)],
                            nTb_t[d][:], start=(d == 0), stop=(d == DT - 1))
                    gt = gT_p.tile([128, 512], BF16, name="gt")
                    nc.scalar.activation(gt[:], ps[:], AF.Gelu_apprx_tanh,
                                         bias=fcb_sb[:, f:f + 1])
                    gT_t.append(gt)
                yield

            # ---- cproj with LN2 stats incrementally ----
            if k == 1:
                dumm = strow_p.tile([1, 3 * 512], F32, tag="st", name="st")
                nc.scalar.activation(dumm[:, 0:1], gT_t[-1][0:1, 0:1],
                                     AF.Sqrt)
            mar_t = []
            pA2, pB2 = ln_stat_begin(psS)
            for p4 in range(4):
                wcq = []
                for qtr in range(4):
                    t_ = wc_p.tile([128, 8 * 256], BF16, name="wcq")
                    nc.sync.dma_start(out=t_[:], in_=wcp[p4, qtr])
                    wcq.append(t_)
                for ci in range(2):
                    dct = 2 * p4 + ci
                    ps = psM.tile([128, 512], F32, tag="mm", name="ps_cp")
                    for f in range(FT):
                        w = wcq[f // 8]
                        fi_ = f % 8
                        nc.tensor.matmul(
                            ps[:],
                            w[:, 256 * fi_ + 128 * ci:256 * fi_ + 128 * (ci + 1)],
                            gT_t[f][:], start=(f == 0), stop=(f == FT - 1))
                    m2 = t1_p.tile([128, 512], F32, name="t1")
                    nc.vector.tensor_add(m2[:], ps[:], nTb_t[dct][:])
                    if use_cprojb:
                        nc.vector.tensor_scalar_add(
                            m2[:], m2[:], cprojb_sb[:, dct:dct + 1])
                    mar_t.append(m2)
                    ln_stat_tile(pA2, pB2, m2, dct)
                yield

            # ---- LN2 -> output ----
            rstd_ps, urstd_ps = ln_stat_finish(pA2, pB2)
            for i in range(DT):
                eng = nc.vector
                ht = hT_p.tile([128, 512], F32, tag="hT", name="ht")
                eng.tensor_mul(ht[:], mar_t[i][:], rstd_ps[:])
                eng.tensor_sub(ht[:], ht[:], urstd_ps[:])
                if use_g2b2:
                    eng.tensor_scalar(ht[:], ht[:], g2_sb[:, i:i + 1],
                                      b2_sb[:, i:i + 1],
                                      OP.mult, OP.add)
                nc.sync.dma_start(out=out[DT * k + i], in_=ht[:])
            yield

        def run(g):
            for _ in g:
                pass

        def trigger_rs(j):
            nc.gpsimd.collective_compute(
                "ReduceScatter", OP.add, replica_groups=PAIRS,
                ins=[rs_in[j][:].opt()], outs=[rs_out[j][:].opt()])

        # =========================== schedule ==============================
        # prologue: QKV(0) as a solid block
        for cost, fn, st in qkv_steps(0):
            fn()

        att_chunk(0, qkv_steps(1))
        att_chunk(1, qkv_steps(2) + proj_steps(0))
        # proj(1) right after att(1) so RS0 can fire early
        for cost, fn, st in proj_steps(1):
            fn()
        trigger_rs(0)
        att_chunk(2, qkv_steps(3))
        t1_b0_box = [None]

        def b0_res_steps():
            def f():
                t1_b0_box[0] = B_res(0)
            return [(0, f, True)]
        att_chunk(3, proj_steps(2) + b0_res_steps())
        # LN1(B0) stats before proj(3): scalar finish chain, stat->bcast
        # latency and the normalize DVE passes all hide under proj(3).
        # tile_wait_until keeps the scheduler from hoisting these RS-gated
        # ops into earlier queue slots where they would block the engine.
        with tc.tile_wait_until(0.255):
            pA1, pB1 = ln_stat_begin(psS)
            for i in range(DT):
                ln_stat_tile(pA1, pB1, t1_b0_box[0][i], i)
        p3 = proj_steps(3, drain_scalar=True)
        for cost, fn, st in p3[:12]:
            fn()
        with tc.tile_wait_until(0.258):
            rstd1, urstd1 = ln_stat_finish(pA1, pB1)
        nTb0 = []

        def norm0(i):
            with tc.tile_wait_until(0.26):
                eng = nc.vector
                tmpn = tmpn_p.tile([128, 512], F32, name="tmpn")
                eng.tensor_mul(tmpn[:], t1_b0_box[0][i][:], rstd1[:])
                nb = nTb_p.tile([128, 512], BF16, tag="nTb", name="nb")
                eng.tensor_sub(nb[:], tmpn[:], urstd1[:])
                if use_g1b1:
                    eng.tensor_scalar(nb[:], nb[:], g1_sb[:, i:i + 1],
                                      b1_sb[:, i:i + 1],
                                      OP.mult, OP.add)
                nTb0.append(nb)
        ni = 0
        for si, (cost, fn, st) in enumerate(p3[12:]):
            fn()
            if si % 2 == 1 and ni < DT:
                norm0(ni)
                ni += 1
        while ni < DT:
            norm0(ni)
            ni += 1
        # release attention weights, open MLP weight pools, then prefetch
        # fc(B0) weights ahead of the collective's DMA traffic
        actx.close()
        wf_p = pool("wf_p", 5)          # fc weight half-groups [128, 2048]
        gT_p = pool("gT_p", 32)         # gelu outputs bf16
        wc_p = pool("wc_p", 3)          # cproj weight quarter [128, 2048]
        wf_pre0 = [load_wf(0), load_wf(1)]
        trigger_rs(1)

        b0 = B(0, nTb0, wf_pre0)
        for _ in range(10):
            next(b0)      # (noop), fc fg0..7, cproj p4-0
        with tc.tile_wait_until(0.41):
            nTb1 = layernorm_to_bf16(B_res(1), g1_sb, b1_sb, use_g1b1, psA)
        b1 = B(1, nTb1)
        next(b1)          # noop yield
        for _ in range(3):
            next(b0)      # cproj p4 1-3
        for _ in range(2):
            next(b1)      # fc fg0, fg1 — cover b0's LN2 finish
        run(b0)           # LN2(B0) finish + normalize + out
        run(b1)           # fc rest, cproj, LN2

    nc.compile()
    return nc


_cache = {}


def _get_program(flags):
    if flags not in _cache:
        _cache[flags] = _build(*flags)
    return _cache[flags]


def _pack(a, rows, cols):
    """[R, C] -> [R//rows * C//cols, rows, cols], row-tile-major."""
    R, C = a.shape
    return np.ascontiguousarray(
        a.reshape(R // rows, rows, C // cols, cols).transpose(0, 2, 1, 3)
        .reshape(-1, rows, cols))


def _swap(p, n_rt, n_ct):
    """_pack gives (row-tile, col-tile) order; swap to (col, row)."""
    t = p.reshape(n_rt, n_ct, p.shape[1], p.shape[2])
    return np.ascontiguousarray(
        t.transpose(1, 0, 2, 3).reshape(-1, p.shape[1], p.shape[2]))


def _rowpack(tiles):
    """[N, 128, C] tile list -> [128, N*C] contiguous-row layout."""
    n, p, c = tiles.shape
    return np.ascontiguousarray(tiles.transpose(1, 0, 2).reshape(p, n * c))


def _prepare_inputs(inputs):
    x = np.asarray(inputs["x"], np.float32)
    weight = float(np.asarray(inputs["weight"]).reshape(-1)[0])
    linear_w = np.asarray(inputs["linear_w"], np.float32)
    linear_b = np.asarray(inputs["linear_b"], np.float32)
    proj_w = np.asarray(inputs["proj_w"], np.float32)
    proj_b = np.asarray(inputs["proj_b"], np.float32)
    ln1_g = np.asarray(inputs["ln1_g"], np.float32)
    ln1_b = np.asarray(inputs["ln1_b"], np.float32)
    fc_w = np.asarray(inputs["fc_w"], np.float32)
    fc_b = np.asarray(inputs["fc_b"], np.float32)
    cproj_w = np.asarray(inputs["cproj_w"], np.float32)
    cproj_b = np.asarray(inputs["cproj_b"], np.float32)
    ln2_g = np.asarray(inputs["ln2_g"], np.float32)
    ln2_b = np.asarray(inputs["ln2_b"], np.float32)
    idx = np.asarray(inputs["idx"]).astype(np.int64).reshape(-1)
    forcing = bool(np.asarray(inputs["attn_forcing"]).reshape(-1)[0])

    flags = (
        bool(linear_b[:2048].any()),      # use_bqk
        bool(linear_b[2048:].any()),      # use_bv
        bool(proj_b.any()),
        bool(cproj_b.any()),
        bool((ln1_g != 1).any() or ln1_b.any()),
        bool((ln2_g != 1).any() or ln2_b.any()),
    )

    if forcing:
        lnw = float(np.log(weight)) if weight >= 1e-37 else -1e9
        pos = np.arange(S)
        lna_all = np.where(pos[None, :] >= idx[:, None], lnw, 0.0) \
            .astype(np.float32)
    else:
        lna_all = np.zeros((B, S), np.float32)

    # strict upper triangle gets -1e9 (causal mask via matmul accumulate):
    # out[k, qq] += trin[qq, k] must be -1e9 when k > qq.
    trin_np = np.where(np.arange(128)[None, :] > np.arange(128)[:, None],
                       np.float32(-1e9), np.float32(0.0)).astype(BF)
    eye_np = np.eye(128, dtype=np.float32).astype(BF)
    sel2_np = np.zeros((1, 256), np.float32)
    sel2_np[0, 0:64] = 1.0       # even-head recip -> partitions 0..63
    sel2_np[0, 128 + 64:] = 1.0  # odd-head recip -> partitions 64..127
    sel2_np = sel2_np.astype(BF)

    # ---- global (all-core) MLP weights ----
    wfc_p = _swap(_pack(fc_w.astype(BF), 128, 512), DT, 8)   # (fg, d)
    wfc_g = np.ascontiguousarray(
        wfc_p.reshape(8, 2, 4, 128, 512).transpose(0, 1, 3, 2, 4)
        .reshape(8, 2, 128, 4 * 512))
    wcp_p = _swap(_pack(cproj_w.astype(BF), 128, 256), FT, 4)  # (p4, f)
    wcp_g = np.ascontiguousarray(
        wcp_p.reshape(4, 4, 8, 128, 256).transpose(0, 1, 3, 2, 4)
        .reshape(4, 4, 128, 8 * 256))

    in_maps = []
    for core in range(N_CORES):
        b, r = core // 2, core % 2
        q_cols = slice(512 * r, 512 * (r + 1))
        k_cols = slice(1024 + 512 * r, 1024 + 512 * (r + 1))
        v_cols = slice(2048 + 512 * r, 2048 + 512 * (r + 1))
        xT = np.ascontiguousarray(x[b].T)                       # [D, S]
        wqk_full = np.concatenate([linear_w[:, q_cols], linear_w[:, k_cols]],
                                  axis=1)                       # [D, 1024]

        xq_t = _pack(xT, 128, 512)                  # (d, c): index d*NCH+c
        xq_dc = xq_t.reshape(DT, NCH, 128, 512)
        xqb = np.ascontiguousarray(
            xq_dc.transpose(1, 0, 2, 3).reshape(NCH, 2, 4, 128, 512)
            .transpose(0, 1, 3, 2, 4).reshape(NCH, 2, 128, 4 * 512)
        ).astype(BF)
        own = [r, 2 + r]
        xo_np = np.ascontiguousarray(
            xq_dc[:, own].transpose(1, 0, 2, 3).reshape(2 * DT, 128, 512))

        in_maps.append({
            "xqb": xqb,
            "xo": xo_np,
            "wqk": _rowpack(_swap(_pack(wqk_full.astype(BF), 128, 512),
                                  8, 2)),
            "bqk": np.ascontiguousarray(
                np.concatenate([linear_b[q_cols], linear_b[k_cols]])),
            "wv": _rowpack(_pack(linear_w[:, v_cols].astype(BF), 128, 512)),
            "bv": np.ascontiguousarray(linear_b[v_cols]).astype(BF),
            "wproj": _rowpack(_swap(_pack(proj_w[512 * r:512 * (r + 1), :]
                                          .astype(BF), 128, 512), 4, 2)),
            "projb": proj_b,
            "wfc": wfc_g, "fcb": fc_b,
            "wcp": wcp_g, "cprojb": cproj_b,
            "g1": ln1_g, "b1": ln1_b, "g2": ln2_g, "b2": ln2_b,
            "lna": lna_all[b],
            "trin": trin_np,
            "eye": eye_np,
            "sel2": sel2_np,
        })
    return flags, in_maps


def _unpack_out(o_pair):
    """o_pair: [out_core0, out_core1] each [2*DT, 128, 512] -> [S, D]."""
    hT = np.empty((D, S), np.float32)
    for r in range(2):
        o = o_pair[r].reshape(2, DT, 128, 512)
        for k, c in enumerate([r, 2 + r]):
            hT[:, CH * c:CH * (c + 1)] = o[k].reshape(D, 512)
    return hT.T


def _run(inputs, trace=False):
    flags, in_maps = _prepare_inputs(inputs)
    nc = _get_program(flags)
    r = run_bass_kernel_spmd(nc, in_maps, core_ids=list(range(N_CORES)),
                             trace=trace)
    outs = np.stack(
        [_unpack_out([r.results[2 * b]["out"], r.results[2 * b + 1]["out"]])
         for b in range(B)], axis=0).astype(np.float32)
    return outs, r


def kernel(**inputs) -> np.ndarray:
    outs, _ = _run(inputs, trace=False)
    return outs
